# revision 1
# baseline (speedup 1.0000x reference)
"""Trainium2 Bass kernel for nn_NeRFGraph (gnn_message_passing).

Strategy (sharding_hint): nodes are sharded across 8 cores aligned to whole
knn batch groups. 20 groups of 2048 nodes -> cores 0-3 take 3 groups,
cores 4-7 take 2 real groups + 1 dummy (SPMD needs uniform shapes; dummy
output is dropped on the host). MLP weights are replicated (data parallel).

Per-core pipeline, per group g (layouts are [features(partitions), nodes(free)]):
  1. kNN via 2 bf16 matmuls per [128,512] score tile (2-term hi/lo split):
     score/2 = x_i.x_j - |x_j|^2/2 with x ~ a+b (bf16 hi/lo):
       MM1: [a;b]_i . [a;a]_j      = a.a + b.a
       MM2: [a;1;1]_i . [b;s1;s2]_j = a.b + (-sq/2 in 2 bf16 terms)
     dropped b.b term ~2e-5 abs -> ~2 neighbor flips / 40960 nodes
     (measured offline; end-to-end impact 1.4e-4 rel). Self always wins
     top-1, so neighbors = entries 1..3 of the DVE max8/max_index scan.
  2. MLP (8 layers + skip at 4) in float32r, processed in node-pair blocks
     of 1024 so PSUM evictions are single [128,1024] Act ops; emitted in
     two half-group batches between kNN row-tiles so PE fills DVE-scan gaps.
  3. EdgeConv x2, factorized: msg_ij = relu(A_i + C_j) with -A (host-negated
     weights) tables; msg_pre = C_gathered - (-A) broadcast on DVE, relu on
     Act (to f32r). C_j gathered with gpsimd ap_gather (per half-group, all
     3 neighbor lists). W2 matmul, mean over K=3 folded into next layer's
     weights (host prescale by 1/3). The whole EdgeConv stage of group g is
     emitted as thunks interleaved into group g+1's kNN/MLP emission, so
     every engine's in-order queue alternates between the two groups
     (software pipeline across groups).
     NOTE: finer-grained MLP interleaving (per-layer drip) corrupts results
     on real HW (rgb rel-err 3e-2) while passing CoreSim bit-exact — kept at
     half-group batches which verify correct on HW.
  4. rgb = sigmoid(S2 @ w_rgb/3 + b_rgb), sigma from the MLP trunk.
"""

import numpy as np
import ml_dtypes

import concourse.bass as bass
import concourse.tile as tile
from concourse import bacc, mybir, library_config
import concourse.bass_utils as bass_utils

F32 = mybir.dt.float32
F32R = mybir.dt.float32r
BF16 = mybir.dt.bfloat16
U16 = mybir.dt.uint16
I16 = mybir.dt.int16
NPBF = ml_dtypes.bfloat16

# problem constants (hardcoded per contract)
B = 40960
NG = 20
DXYZ = 63
DDIR = 27
W = 256
H = 128  # W // 2
KNN = 3

N_CORES = 8
GPC = 3                      # groups per core (SPMD-uniform)
G = B // NG                  # 2048 nodes per group
NODES = GPC * G              # 6144 nodes per core
MT = G // 128                # row tiles of 128 per group (knn)
NP = G // 1024               # node pairs of 1024 per group (mlp)

_STATE: dict = {}


def _build_nc(reps=1):
    nc = bacc.Bacc(
        "TRN2",
        target_bir_lowering=False,
        debug=False,
        enable_asserts=False,
        num_devices=N_CORES,
    )
    d = {}

    def inp(name, shape, dt=F32):
        d[name] = nc.dram_tensor(name, list(shape), dt, kind="ExternalInput").ap()

    inp("xt", (91, NODES), F32R)    # rows 0-62 xyz, 63 ones, 64-90 dir (f32 bits)
    inp("kl", (126, NODES), BF16)   # [a; b]      stationary MM1
    inp("ks", (66, NODES), BF16)    # [a; 1; 1; 1] stationary MM2 (pad to 66)
    inp("ma", (126, NODES), BF16)   # [a; a]      moving MM1
    inp("mb", (66, NODES), BF16)    # [b; s1; s2; 0] moving MM2
    inp("w0", (DXYZ, W)); inp("b0", (W, 1))
    inp("wmid", (6, W, W))          # [layer, in, out]
    inp("bmid", (6, W, 1))
    inp("wskip", (DXYZ + W, W)); inp("bskip", (W, 1))
    inp("wfin", (W, W)); inp("bfin", (W, 1))
    inp("wsig", (W, 1)); inp("bsig", (1, 1))
    inp("naw1", (W + DDIR, H)); inp("cw1", (W + DDIR, H)); inp("nab1", (H, 1))
    inp("e1w2", (H, H)); inp("e1b2", (H, 1))
    inp("na2w", (H, H)); inp("c2w", (H, H)); inp("nab2", (H, 1))
    inp("e2w2", (H, H)); inp("e2b2", (H, 1))
    inp("wrgb", (H, 3)); inp("brgb", (3, 1))

    rgb_d = nc.dram_tensor("rgb", [3, NODES], F32, kind="ExternalOutput").ap()
    sig_d = nc.dram_tensor("sig", [1, NODES], F32, kind="ExternalOutput").ap()

    with tile.TileContext(nc) as tc:
        _body(tc, d, rgb_d, sig_d, reps=reps)
    nc.compile()
    return nc


def _body(tc, d, rgb_d, sig_d, reps=1):
    nc = tc.nc
    ctxs = []

    def pool(name, bufs, space="SBUF"):
        p = tc.tile_pool(name=name, bufs=bufs, space=space)
        ctxs.append(p)
        return p.__enter__()

    wstage = pool("wstage", bufs=1)       # f32 staging for weight rounding
    wp = pool("wp", bufs=1)               # persistent rounded weights / biases
    xp = pool("xp", bufs=2)               # per-group xt (f32r)
    xk = pool("xk", bufs=1)               # knn bf16 inputs
    ap2 = pool("ap2", bufs=2)             # per-group na1/c1 gather tables
    ap1 = pool("ap1", bufs=1)             # per-group s1/na2/c2 tables
    ec = pool("ec", bufs=2)               # edge-conv small chunk tiles
    ecb = pool("ecb", bufs=2)             # edge-conv big msg tiles (Pool-only)
    ecg = pool("ecg", bufs=1)             # full-group gather outputs [128,3,G]
    hp = pool("hp", bufs=2)               # MLP hidden tiles [128,1024]
    fp = pool("fp", bufs=1)               # feat tiles [128,1024]
    sp = pool("sp", bufs=4)               # small tiles (vmax/imax)
    op = pool("op", bufs=1)               # output staging
    idxp = pool("idxp", bufs=2)
    psp = pool("psp", bufs=1, space="PSUM")    # knn scores [128,2048]
    psm = pool("psm", bufs=2, space="PSUM")    # everything else [128,1024]
    drp = pool("drp", bufs=2, space="DRAM")

    nc.gpsimd.load_library(library_config.ap_gather)

    # ---- per-group input loads (group 0 emitted BEFORE the weight loads so
    # its DMAs are first in the HWDGE queues and kNN can start immediately) ----
    def load_group(g):
        g0 = g * G
        t = {}
        t["xt"] = xp.tile([91, G], F32R, tag="xt", name="xt")
        nc.sync.dma_start(t["xt"][:], d["xt"][:, g0:g0 + G])
        for nm, rows in (("kl", 126), ("ks", 66), ("ma", 126), ("mb", 66)):
            t[nm] = xk.tile([rows, G], BF16, tag=nm, name=nm)
            nc.sync.dma_start(t[nm][:], d[nm][:, g0:g0 + G])
        return t

    _pre0 = load_group(0)

    # ---- load + round weights to f32r (one-time) ----
    def load_chunks(src_ap, rows, cols, tag, part_off=0):
        """src_ap: DRAM AP [R, cols]; returns list of rounded chunk tile APs.
        part_off: place the LAST chunk at this base partition (32-aligned)."""
        out = []
        r0 = 0
        for i, r in enumerate(rows):
            last = i == len(rows) - 1
            if last and part_off:
                st = wstage.tile([part_off + r, cols], F32, tag="wstage_p", name="stp")
                nc.sync.dma_start(st[part_off:part_off + r, :], src_ap[r0:r0 + r, :])
                wt = wp.tile([part_off + r, cols], F32R, tag=f"{tag}_{i}", name="wtp")
                nc.scalar.activation(wt[part_off:part_off + r, :],
                                     st[part_off:part_off + r, :],
                                     mybir.ActivationFunctionType.Identity)
                out.append(wt[part_off:part_off + r, :])
            else:
                st = wstage.tile([r, cols], F32, tag="wstage", name="st")
                nc.sync.dma_start(st[:], src_ap[r0:r0 + r, :])
                wt = wp.tile([r, cols], F32R, tag=f"{tag}_{i}", name="wt")
                nc.scalar.activation(wt[:], st[:], mybir.ActivationFunctionType.Identity)
                out.append(wt[:])
            r0 += r
        return out

    def load_b(name, src_ap, rows):
        out = []
        r0 = 0
        for i, r in enumerate(rows):
            bt = wp.tile([r, 1], F32, tag=f"{name}_{i}", name="bt")
            nc.sync.dma_start(bt[:], src_ap[r0:r0 + r, :])
            out.append(bt)
            r0 += r
        return out

    w0 = load_chunks(d["w0"][:], [DXYZ], W, "w0")[0]
    wmid = [load_chunks(d["wmid"][l], [128, 128], W, f"wmid{l}") for l in range(6)]
    wskip = load_chunks(d["wskip"][:], [DXYZ, 128, 128], W, "wskip")
    wfin = load_chunks(d["wfin"][:], [128, 128], W, "wfin")
    wsig = load_chunks(d["wsig"][:], [128, 128], 1, "wsig")
    # dir chunk placed at base partition 64 to match xt[64:91] (32-aligned)
    naw1 = load_chunks(d["naw1"][:], [128, 128, DDIR], H, "naw1", part_off=64)
    cw1 = load_chunks(d["cw1"][:], [128, 128, DDIR], H, "cw1", part_off=64)
    e1w2 = load_chunks(d["e1w2"][:], [H], H, "e1w2")[0]
    na2w = load_chunks(d["na2w"][:], [H], H, "na2w")[0]
    c2w = load_chunks(d["c2w"][:], [H], H, "c2w")[0]
    e2w2 = load_chunks(d["e2w2"][:], [H], H, "e2w2")[0]
    wrgb = load_chunks(d["wrgb"][:], [H], 3, "wrgb")[0]

    b0 = load_b("b0", d["b0"][:], [128, 128])
    bmid = [load_b(f"bmid{l}", d["bmid"][l], [128, 128]) for l in range(6)]
    bskip = load_b("bskip", d["bskip"][:], [128, 128])
    bfin = load_b("bfin", d["bfin"][:], [128, 128])
    bsig = load_b("bsig", d["bsig"][:], [1])[0]
    nab1 = load_b("nab1", d["nab1"][:], [H])[0]
    e1b2 = load_b("e1b2", d["e1b2"][:], [H])[0]
    nab2 = load_b("nab2", d["nab2"][:], [H])[0]
    e2b2 = load_b("e2b2", d["e2b2"][:], [H])[0]
    brgb = load_b("brgb", d["brgb"][:], [3])[0]

    ACT = mybir.ActivationFunctionType

    pending = []
    for gi in range(GPC * reps):
        g = gi % GPC
        g0 = g * G
        t_in = _pre0 if gi == 0 else load_group(g)
        xt = t_in["xt"]
        kl = t_in["kl"]; ks = t_in["ks"]; ma = t_in["ma"]; mb = t_in["mb"]

        nbr = drp.tile([KNN * G], U16, tag="nbr")   # wrapped k-major image
        # addr = k*2048 + r*128 + q  <->  element m of gather list k at [r=m%16, f=m//16]
        nbr3 = nbr[:].rearrange("(k r q) -> q r k", k=KNN, r=16, q=128)
        na1 = ap2.tile([H, G], F32, tag="na1")
        c1 = ap2.tile([H, G], F32, tag="c1")

        def knn_mt(mt):
            ps = psp.tile([128, 2048], F32, tag="ps", name="ps")
            msl = slice(mt * 128, (mt + 1) * 128)
            for nt in range(4):
                osl = slice(nt * 512, (nt + 1) * 512)
                nc.tensor.matmul(ps[:, osl], kl[:, msl], ma[:, osl],
                                 start=True, stop=False)
            for nt in range(4):
                osl = slice(nt * 512, (nt + 1) * 512)
                nc.tensor.matmul(ps[:, osl], ks[:, msl], mb[:, osl],
                                 start=False, stop=True)
            vmax = sp.tile([128, 8], F32, tag="vmax")
            nc.vector.max(vmax[:], ps[:])
            imax = sp.tile([128, 8], U16, tag="imax")
            nc.vector.max_index(imax[:], vmax[:], ps[:])
            for k in range(KNN):
                nc.sync.dma_start(nbr3[mt * 8:(mt + 1) * 8, :, k], imax[:, 1 + k])

        def mlp_np_thunks(p):
            """MLP trunk for node-pair block p, as a list of per-layer thunks
            so PE work can be dripped between kNN row-tiles."""
            n0 = p * 1024
            sl = slice(n0, n0 + 1024)
            hsl = [slice(n0, n0 + 512), slice(n0 + 512, n0 + 1024)]
            lsl = [slice(0, 512), slice(512, 1024)]
            st = {}

            def layer_mms(ps_list, wchunks, movers):
                # ps_list: [tile for ch0, tile for ch1]; movers: list of
                # (moving AP for ns0, moving AP for ns1) per weight chunk
                for ch in range(2):
                    csl = slice(ch * 128, (ch + 1) * 128)
                    for ns in range(2):
                        for i, wk in enumerate(wchunks):
                            nc.tensor.matmul(
                                ps_list[ch][:, lsl[ns]], wk[:, csl], movers[i][ns],
                                start=(i == 0), stop=(i == len(wchunks) - 1))

            def hmov(h):
                return [(h[0][:, lsl[0]], h[0][:, lsl[1]]),
                        (h[1][:, lsl[0]], h[1][:, lsl[1]])]

            def t0():
                ps = [psm.tile([128, 1024], F32, tag="pm", name=f"ps0{ch}") for ch in range(2)]
                layer_mms(ps, [w0], [(xt[0:DXYZ, hsl[0]], xt[0:DXYZ, hsl[1]])])
                h = [hp.tile([128, 1024], F32R, tag=f"h{ch}", name=f"h{ch}") for ch in range(2)]
                for ch in range(2):
                    nc.scalar.activation(h[ch][:], ps[ch][:], ACT.Relu, bias=b0[ch][:])
                st["h"] = h

            def tl(layer, m):
                h = st["h"]
                ps = [psm.tile([128, 1024], F32, tag="pm", name=f"psl{ch}") for ch in range(2)]
                if layer == 4:
                    bk = bskip
                    layer_mms(ps, wskip,
                              [(xt[0:DXYZ, hsl[0]], xt[0:DXYZ, hsl[1]])] + hmov(h))
                else:
                    bk = bmid[m]
                    layer_mms(ps, wmid[m], hmov(h))
                hn = [hp.tile([128, 1024], F32R, tag=f"h{ch}", name=f"hn{ch}") for ch in range(2)]
                for ch in range(2):
                    nc.scalar.activation(hn[ch][:], ps[ch][:], ACT.Relu, bias=bk[ch][:])
                st["h"] = hn

            def tfin():
                movers = hmov(st["h"])
                ps = [psm.tile([128, 1024], F32, tag="pm", name=f"psf{ch}") for ch in range(2)]
                layer_mms(ps, wfin, movers)
                pss = psm.tile([1, 1024], F32, tag="pm", name="pss")
                for ns in range(2):
                    nc.tensor.matmul(pss[0:1, lsl[ns]], wsig[0][:], movers[0][ns],
                                     start=True, stop=False)
                    nc.tensor.matmul(pss[0:1, lsl[ns]], wsig[1][:], movers[1][ns],
                                     start=False, stop=True)
                feat = [fp.tile([128, 1024], F32R, tag=f"feat{ch}", name=f"feat{ch}") for ch in range(2)]
                for ch in range(2):
                    nc.scalar.activation(feat[ch][:], ps[ch][:], ACT.Identity, bias=bfin[ch][:])
                sgt = op.tile([1, 1024], F32, tag="sgt")
                nc.scalar.activation(sgt[:], pss[0:1, :], ACT.Identity, bias=bsig[:])
                nc.sync.dma_start(sig_d[:, g0 + n0:g0 + n0 + 1024], sgt[:])
                st["feat"] = feat

            def ta1c1():
                feat = st["feat"]
                fmov = hmov(feat) + [(xt[64:91, hsl[0]], xt[64:91, hsl[1]])]
                psA = psm.tile([128, 1024], F32, tag="pm", name="psA")
                psC = psm.tile([128, 1024], F32, tag="pm", name="psC")
                for ns in range(2):
                    for i in range(3):
                        nc.tensor.matmul(psA[:, lsl[ns]], naw1[i], fmov[i][ns],
                                         start=(i == 0), stop=(i == 2))
                for ns in range(2):
                    for i in range(3):
                        nc.tensor.matmul(psC[:, lsl[ns]], cw1[i], fmov[i][ns],
                                         start=(i == 0), stop=(i == 2))
                nc.scalar.activation(na1[:, sl], psA[:], ACT.Identity, bias=nab1[:])
                nc.scalar.activation(c1[:, sl], psC[:], ACT.Copy)

            thunks = [t0]
            m = 0
            for layer in range(1, 8):
                mm = m
                thunks.append(lambda l=layer, mi_=mm: tl(l, mi_))
                if layer != 4:
                    m += 1
            thunks += [tfin, ta1c1]
            return thunks

        # interleave knn row-tiles + MLP layer thunks + the PREVIOUS group's
        # EdgeConv thunks, so every engine's in-order queue alternates
        # between the two groups (software pipeline) and PE work is smooth.
        mlp_thunks = mlp_np_thunks(0) + mlp_np_thunks(1)
        ti = 0
        mi = 0
        nmt = len(mlp_thunks)
        for mt in range(MT):
            knn_mt(mt)
            if ti < len(pending):
                pending[ti](); ti += 1
            if mt % 8 == 7:
                while mi < (mt + 1) * nmt // MT:
                    mlp_thunks[mi](); mi += 1
                if ti < len(pending):
                    pending[ti](); ti += 1
        while mi < nmt:
            mlp_thunks[mi](); mi += 1
        while ti < len(pending):
            pending[ti](); ti += 1

        # wrapped gather indices, k-major: idxw[:, k*128+f] block for neighbor k.
        # One strided DRAM read into partitions 0:16, then replicate to all
        # 16-partition blocks (one per Q7 core) with SBUF->SBUF copies.
        idxw = idxp.tile([128, G * KNN // 16], I16, tag="idxw")
        nbr_r = nbr[:].rearrange("(k r f) -> r k f", k=KNN, r=16, f=128)
        nc.sync.dma_start(
            idxw[0:16, :].rearrange("r (k f) -> r k f", k=KNN),
            nbr_r.bitcast(I16))
        for r in range(1, 8):
            nc.sync.dma_start(idxw[16 * r:16 * r + 16, :], idxw[0:16, :])

        def make_ec_thunks(g0, idxw, na1, c1):
            """Build the EdgeConv thunk list for this group; emitted later,
            interleaved into the NEXT group's knn/mlp emission."""
            s1 = ap1.tile([H, G], F32R, tag="s1")
            na2 = ap1.tile([H, G], F32, tag="na2")
            c2 = ap1.tile([H, G], F32, tag="c2")
            thunks = []

            def conv_thunks(src, nA, w2, b2, dst_of, out_cb):
                gts = {}

                def gather_half(half):
                    gt = ecg.tile([128, KNN, 1024], F32, tag="g1")
                    gts[half] = gt
                    for k in range(KNN):
                        nc.gpsimd.ap_gather(
                            gt[:, k, :], src[:],
                            idxw[:, k * 128 + half * 64:k * 128 + half * 64 + 64],
                            channels=128, num_elems=G, d=1, num_idxs=1024)

                def do_chunk(c):
                    gt, cc = gts[c // 2], c % 2
                    nsl = slice(c * 512, (c + 1) * 512)
                    nab = nA[:, nsl].unsqueeze(1).to_broadcast([H, KNN, 512])
                    msgp = ecb.tile([128, KNN, 512], F32, tag="tmx")
                    nc.vector.tensor_sub(msgp[:], gt[:, :, cc * 512:(cc + 1) * 512], nab)
                    msgr = ecb.tile([128, KNN, 512], F32R, tag="msgr")
                    nc.scalar.activation(msgr[:], msgp[:], ACT.Relu)
                    mr = msgr[:]
                    psE = psm.tile([128, 1024], F32, tag="pm", name="psE")
                    psE2 = psm.tile([128, 1024], F32, tag="pm", name="psE2")
                    nc.tensor.matmul(psE[:, 0:512], w2[:], mr[:, 0, :], start=True, stop=True)
                    nc.tensor.matmul(psE[:, 512:1024], w2[:], mr[:, 1, :], start=True, stop=True)
                    nc.tensor.matmul(psE2[:, 0:512], w2[:], mr[:, 2, :], start=True, stop=True)
                    h2 = ec.tile([128, 1024], F32, tag="h2")
                    h22 = ec.tile([128, 512], F32, tag="h22")
                    nc.scalar.activation(h2[:], psE[:], ACT.Relu, bias=b2[:])
                    nc.scalar.activation(h22[:], psE2[:, 0:512], ACT.Relu, bias=b2[:])
                    tmp = ec.tile([128, 512], F32, tag="trio")
                    nc.vector.tensor_add(tmp[:], h2[:, 0:512], h2[:, 512:1024])
                    dst = dst_of(c)
                    nc.vector.tensor_add(dst, tmp[:], h22[:])
                    out_cb(c, dst)

                for half in range(2):
                    thunks.append(lambda h=half: gather_half(h))
                    thunks.append(lambda c=half * 2: do_chunk(c))
                    thunks.append(lambda c=half * 2 + 1: do_chunk(c))

            # ---- EdgeConv 1 ----
            conv_thunks(c1, na1, e1w2, e1b2,
                        lambda c: s1[:, c * 512:(c + 1) * 512], lambda c, dstap: None)

            # ---- A2 / C2 ----
            def a2c2(p):
                lsl = [slice(p * 1024, p * 1024 + 512), slice(p * 1024 + 512, p * 1024 + 1024)]
                s1r = s1[:]
                psA = psm.tile([128, 1024], F32, tag="pm", name="psA2")
                psC = psm.tile([128, 1024], F32, tag="pm", name="psC2")
                for ns in range(2):
                    nc.tensor.matmul(psA[:, ns * 512:(ns + 1) * 512], na2w[:], s1r[:, lsl[ns]],
                                     start=True, stop=True)
                    nc.tensor.matmul(psC[:, ns * 512:(ns + 1) * 512], c2w[:], s1r[:, lsl[ns]],
                                     start=True, stop=True)
                nc.scalar.activation(na2[:, p * 1024:(p + 1) * 1024], psA[:], ACT.Identity, bias=nab2[:])
                nc.scalar.activation(c2[:, p * 1024:(p + 1) * 1024], psC[:], ACT.Copy)

            thunks.append(lambda: a2c2(0))
            thunks.append(lambda: a2c2(1))

            # ---- EdgeConv 2 ----
            s2t = ec.tile([128, 512], F32R, tag="s2final")

            def ec2_out(c, dstap):
                psR = psm.tile([3, 512], F32, tag="pm", name="psR")
                nc.tensor.matmul(psR[0:3, 0:512], wrgb[:], dstap,
                                 start=True, stop=True)
                rgt = op.tile([3, 512], F32, tag="rgt")
                nc.scalar.activation(rgt[:], psR[0:3, 0:512], ACT.Sigmoid, bias=brgb[:])
                nc.sync.dma_start(rgb_d[:, g0 + c * 512:g0 + (c + 1) * 512], rgt[:])

            conv_thunks(c2, na2, e2w2, e2b2, lambda c: s2t[:], ec2_out)
            return thunks

        pending = make_ec_thunks(g0, idxw, na1, c1)

    for t in pending:
        t()

    for p in reversed(ctxs):
        p.__exit__(None, None, None)


def _core_groups():
    cg = []
    for c in range(N_CORES):
        if c < 4:
            gs = [3 * c, 3 * c + 1, 3 * c + 2]
        else:
            g0 = 12 + 2 * (c - 4)
            gs = [g0, g0 + 1, g0]  # 3rd slot = dummy repeat
        cg.append(gs)
    return cg


def _prep(inputs):
    x = np.asarray(inputs["x"], dtype=np.float32)
    batch_ids = np.asarray(inputs["batch_ids"])
    perm = np.argsort(batch_ids, kind="stable")
    xs = np.ascontiguousarray(x[perm])

    xyz = xs[:, :DXYZ]
    sq = (xyz * xyz).sum(1, dtype=np.float32)

    w = {k: np.asarray(inputs[k], dtype=np.float32) for k in inputs if k not in ("x", "batch_ids")}
    e1 = w["e1_w1"]
    naw1 = np.ascontiguousarray(-(e1[:W + DDIR] - e1[W + DDIR:]))
    cw1 = np.ascontiguousarray(e1[W + DDIR:])
    e2 = w["e2_w1"]
    na2w = np.ascontiguousarray(-(e2[:H] - e2[H:]) / 3.0)
    c2w = np.ascontiguousarray(e2[H:] / 3.0)

    shared = {
        "w0": w["w0"], "b0": w["b0"].reshape(W, 1),
        "wmid": w["w_mid"], "bmid": w["b_mid"].reshape(6, W, 1),
        "wskip": w["w_skip"], "bskip": w["b_skip"].reshape(W, 1),
        "wfin": w["w_final"], "bfin": w["b_final"].reshape(W, 1),
        "wsig": w["w_sigma"], "bsig": w["b_sigma"].reshape(1, 1),
        "naw1": naw1, "cw1": cw1, "nab1": -w["e1_b1"].reshape(H, 1),
        "e1w2": w["e1_w2"], "e1b2": w["e1_b2"].reshape(H, 1),
        "na2w": na2w, "c2w": c2w, "nab2": -w["e2_b1"].reshape(H, 1),
        "e2w2": w["e2_w2"], "e2b2": w["e2_b2"].reshape(H, 1),
        "wrgb": np.ascontiguousarray(w["w_rgb"] / 3.0), "brgb": w["b_rgb"].reshape(3, 1),
    }
    shared = {k: np.ascontiguousarray(v, dtype=np.float32) for k, v in shared.items()}

    in_maps = []
    for gs in _core_groups():
        rows = np.concatenate([np.arange(g * G, (g + 1) * G) for g in gs])
        xc = xs[rows]
        xyzT = np.ascontiguousarray(xc[:, :DXYZ].T)   # [63, NODES] f32
        xt = np.empty((91, NODES), np.float32)
        xt[0:DXYZ] = xyzT
        xt[DXYZ] = 1.0
        xt[DXYZ + 1:] = xc[:, DXYZ:].T

        a = xyzT.astype(NPBF)
        bb = (xyzT - a.astype(np.float32)).astype(NPBF)
        nh = -0.5 * sq[rows]
        s1 = nh.astype(NPBF)
        s2 = (nh - s1.astype(np.float32)).astype(NPBF)
        ones2 = np.ones((3, NODES), NPBF)
        zeros1 = np.zeros((1, NODES), NPBF)

        m = dict(shared)
        m["xt"] = np.ascontiguousarray(xt)
        m["kl"] = np.ascontiguousarray(np.concatenate([a, bb], 0))
        m["ks"] = np.ascontiguousarray(np.concatenate([a, ones2], 0))
        m["ma"] = np.ascontiguousarray(np.concatenate([a, a], 0))
        m["mb"] = np.ascontiguousarray(np.concatenate([bb, s1[None], s2[None], zeros1], 0))
        in_maps.append(m)
    return in_maps, perm


def _assemble(results, perm):
    out_sorted = np.empty((B, 4), np.float32)
    for c, gs in enumerate(_core_groups()):
        r = results[c]
        for slot, g in enumerate(gs):
            if c >= 4 and slot == 2:
                continue  # dummy
            sl = slice(slot * G, (slot + 1) * G)
            out_sorted[g * G:(g + 1) * G, 0:3] = r["rgb"][:, sl].T
            out_sorted[g * G:(g + 1) * G, 3] = r["sig"][0, sl]
    out = np.empty((B, 4), np.float32)
    out[perm] = out_sorted
    return out


def get_nc(reps=1):
    key = f"nc{reps}"
    if key not in _STATE:
        _STATE[key] = _build_nc(reps)
    return _STATE[key]


def kernel(**inputs) -> np.ndarray:
    nc = get_nc()
    in_maps, perm = _prep(inputs)
    res = bass_utils.run_bass_kernel_spmd(nc, in_maps, core_ids=list(range(N_CORES)))
    return _assemble(res.results, perm)



# revision 6
# speedup vs baseline: 2.7567x; 2.7567x over previous
"""Trainium2 Bass kernel for nn_NeRFGraph (gnn_message_passing).

Strategy (sharding_hint): nodes are sharded across 8 cores aligned to whole
knn batch groups. 20 groups of 2048 nodes -> cores 0-3 take 3 groups,
cores 4-7 take 2 real groups + 1 dummy (SPMD needs uniform shapes; dummy
output is dropped on the host). MLP weights are replicated (data parallel).

Per-core pipeline, per group g (layouts are [features(partitions), nodes(free)]):
  1. kNN via 2 bf16 matmuls per [128,512] score tile (2-term hi/lo split):
     score/2 = x_i.x_j - |x_j|^2/2 with x ~ a+b (bf16 hi/lo):
       MM1: [a;b]_i . [a;a]_j      = a.a + b.a
       MM2: [a;1;1]_i . [b;s1;s2]_j = a.b + (-sq/2 in 2 bf16 terms)
     dropped b.b term ~2e-5 abs -> ~2 neighbor flips / 40960 nodes
     (measured offline; end-to-end impact 1.4e-4 rel). Self always wins
     top-1, so neighbors = entries 1..3 of the DVE max8/max_index scan.
  2. MLP (8 layers + skip at 4) in float32r, processed in node-pair blocks
     of 1024 so PSUM evictions are single [128,1024] Act ops; emitted in
     two half-group batches between kNN row-tiles so PE fills DVE-scan gaps.
  3. EdgeConv x2, factorized: msg_ij = relu(A_i + C_j) with -A (host-negated
     weights) tables; msg_pre = C_gathered - (-A) broadcast on DVE, relu on
     Act (to f32r). C_j gathered with gpsimd ap_gather (per half-group, all
     3 neighbor lists). W2 matmul, mean over K=3 folded into next layer's
     weights (host prescale by 1/3). The whole EdgeConv stage of group g is
     emitted as thunks interleaved into group g+1's kNN/MLP emission, so
     every engine's in-order queue alternates between the two groups
     (software pipeline across groups).
     NOTE: finer-grained MLP interleaving (per-layer drip) corrupts results
     on real HW (rgb rel-err 3e-2) while passing CoreSim bit-exact — kept at
     half-group batches which verify correct on HW.
  4. rgb = sigmoid(S2 @ w_rgb/3 + b_rgb), sigma from the MLP trunk.
"""

import numpy as np
import ml_dtypes

import concourse.bass as bass
import concourse.tile as tile
from concourse import bacc, mybir, library_config
import concourse.bass_utils as bass_utils

F32 = mybir.dt.float32
F32R = mybir.dt.float32r
BF16 = mybir.dt.bfloat16
U16 = mybir.dt.uint16
I16 = mybir.dt.int16
NPBF = ml_dtypes.bfloat16

# problem constants (hardcoded per contract)
B = 40960
NG = 20
DXYZ = 63
DDIR = 27
W = 256
H = 128  # W // 2
KNN = 3

N_CORES = 8
GPC = 3                      # groups per core (SPMD-uniform)
G = B // NG                  # 2048 nodes per group
NODES = GPC * G              # 6144 nodes per core
MT = G // 128                # row tiles of 128 per group (knn)
NP = G // 1024               # node pairs of 1024 per group (mlp)

_STATE: dict = {}


def _build_nc(reps=1):
    nc = bacc.Bacc(
        "TRN2",
        target_bir_lowering=False,
        debug=False,
        enable_asserts=False,
        num_devices=N_CORES,
        num_swdge_queues=4,
    )
    d = {}

    def inp(name, shape, dt=F32):
        d[name] = nc.dram_tensor(name, list(shape), dt, kind="ExternalInput").ap()

    inp("xt", (91, NODES), F32R)    # rows 0-62 xyz, 63 ones, 64-90 dir (f32 bits)
    inp("kl", (126, NODES), BF16)   # [a; b]      stationary MM1
    inp("ks", (66, NODES), BF16)    # [a; 1; 1; 1] stationary MM2 (pad to 66)
    inp("ma", (126, NODES), BF16)   # [a; a]      moving MM1
    inp("mb", (66, NODES), BF16)    # [b; s1; s2; 0] moving MM2
    inp("w0", (DXYZ, W)); inp("b0", (W, 1))
    inp("wmid", (6, W, W))          # [layer, in, out]
    inp("bmid", (6, W, 1))
    inp("wskip", (DXYZ + W, W)); inp("bskip", (W, 1))
    inp("wfin", (W, W)); inp("bfin", (W, 1))
    inp("wsig", (W, 1)); inp("bsig", (1, 1))
    inp("naw1", (W + DDIR, H)); inp("cw1", (W + DDIR, H)); inp("nab1", (H, 1))
    inp("e1w2", (H, H)); inp("e1b2", (H, 1))
    inp("na2w", (H, H)); inp("c2w", (H, H)); inp("nab2", (H, 1))
    inp("e2w2", (H, H)); inp("e2b2", (H, 1))
    inp("wrgb", (H, 3)); inp("brgb", (3, 1))

    rgb_d = nc.dram_tensor("rgb", [3, NODES], F32, kind="ExternalOutput").ap()
    sig_d = nc.dram_tensor("sig", [1, NODES], F32, kind="ExternalOutput").ap()

    with tile.TileContext(nc) as tc:
        _body(tc, d, rgb_d, sig_d, reps=reps)
    nc.compile()
    return nc


def _body(tc, d, rgb_d, sig_d, reps=1):
    nc = tc.nc
    ctxs = []

    def pool(name, bufs, space="SBUF"):
        p = tc.tile_pool(name=name, bufs=bufs, space=space)
        ctxs.append(p)
        return p.__enter__()

    wstage = pool("wstage", bufs=1)       # f32 staging for weight rounding
    wp = pool("wp", bufs=1)               # persistent rounded weights / biases
    xp = pool("xp", bufs=2)               # per-group xt (f32r)
    xk = pool("xk", bufs=1)               # knn bf16 inputs
    ap2 = pool("ap2", bufs=2)             # per-group na1/c1 gather tables
    ap1 = pool("ap1", bufs=1)             # per-group s1/na2/c2 tables
    ec = pool("ec", bufs=2)               # edge-conv small chunk tiles
    ecb = pool("ecb", bufs=2)             # edge-conv big msg tiles (Pool-only)
    ecg = pool("ecg", bufs=1)             # full-group gather outputs [128,3,G]
    hp = pool("hp", bufs=2)               # MLP hidden tiles [128,1024]
    fp = pool("fp", bufs=1)               # feat tiles [128,1024]
    sp = pool("sp", bufs=4)               # small tiles (vmax/imax)
    op = pool("op", bufs=1)               # output staging
    idxp = pool("idxp", bufs=2)
    psp = pool("psp", bufs=1, space="PSUM")    # knn scores [128,2048]
    psm = pool("psm", bufs=2, space="PSUM")    # everything else [128,1024]
    drp = pool("drp", bufs=2, space="DRAM")

    nc.gpsimd.load_library(library_config.mlp)
    # SWDGE queue assignment must match Tile's DMASW sem rotation (8 sems,
    # round-robin): queue = (emission index of Pool-engine DMA insts) % 4.
    swq = [0]

    def nextq():
        q = swq[0] % 4
        swq[0] += 1
        return q

    # ---- per-group input loads (group 0 emitted BEFORE the weight loads so
    # its DMAs are first in the HWDGE queues and kNN can start immediately) ----
    def load_group(g):
        g0 = g * G
        t = {}
        t["xt"] = xp.tile([91, G], F32R, tag="xt", name="xt")
        nc.sync.dma_start(t["xt"][:], d["xt"][:, g0:g0 + G])
        for nm, rows in (("kl", 126), ("ks", 66), ("ma", 126), ("mb", 66)):
            t[nm] = xk.tile([rows, G], BF16, tag=nm, name=nm)
            nc.sync.dma_start(t[nm][:], d[nm][:, g0:g0 + G])
        return t

    _pre0 = load_group(0)

    # ---- load + round weights to f32r (one-time) ----
    def load_chunks(src_ap, rows, cols, tag, part_off=0):
        """src_ap: DRAM AP [R, cols]; returns list of rounded chunk tile APs.
        part_off: place the LAST chunk at this base partition (32-aligned)."""
        out = []
        r0 = 0
        for i, r in enumerate(rows):
            last = i == len(rows) - 1
            if last and part_off:
                st = wstage.tile([part_off + r, cols], F32, tag="wstage_p", name="stp")
                nc.sync.dma_start(st[part_off:part_off + r, :], src_ap[r0:r0 + r, :])
                wt = wp.tile([part_off + r, cols], F32R, tag=f"{tag}_{i}", name="wtp")
                nc.scalar.activation(wt[part_off:part_off + r, :],
                                     st[part_off:part_off + r, :],
                                     mybir.ActivationFunctionType.Identity)
                out.append(wt[part_off:part_off + r, :])
            else:
                st = wstage.tile([r, cols], F32, tag="wstage", name="st")
                nc.sync.dma_start(st[:], src_ap[r0:r0 + r, :])
                wt = wp.tile([r, cols], F32R, tag=f"{tag}_{i}", name="wt")
                nc.scalar.activation(wt[:], st[:], mybir.ActivationFunctionType.Identity)
                out.append(wt[:])
            r0 += r
        return out

    def load_b(name, src_ap, rows):
        out = []
        r0 = 0
        for i, r in enumerate(rows):
            bt = wp.tile([r, 1], F32, tag=f"{name}_{i}", name="bt")
            nc.sync.dma_start(bt[:], src_ap[r0:r0 + r, :])
            out.append(bt)
            r0 += r
        return out

    w0 = load_chunks(d["w0"][:], [DXYZ], W, "w0")[0]
    wmid = [load_chunks(d["wmid"][l], [128, 128], W, f"wmid{l}") for l in range(6)]
    wskip = load_chunks(d["wskip"][:], [DXYZ, 128, 128], W, "wskip")
    wfin = load_chunks(d["wfin"][:], [128, 128], W, "wfin")
    wsig = load_chunks(d["wsig"][:], [128, 128], 1, "wsig")
    # dir chunk placed at base partition 64 to match xt[64:91] (32-aligned)
    naw1 = load_chunks(d["naw1"][:], [128, 128, DDIR], H, "naw1", part_off=64)
    cw1 = load_chunks(d["cw1"][:], [128, 128, DDIR], H, "cw1", part_off=64)
    e1w2 = load_chunks(d["e1w2"][:], [H], H, "e1w2")[0]
    na2w = load_chunks(d["na2w"][:], [H], H, "na2w")[0]
    c2w = load_chunks(d["c2w"][:], [H], H, "c2w")[0]
    e2w2 = load_chunks(d["e2w2"][:], [H], H, "e2w2")[0]
    wrgb = load_chunks(d["wrgb"][:], [H], 3, "wrgb")[0]

    b0 = load_b("b0", d["b0"][:], [128, 128])
    bmid = [load_b(f"bmid{l}", d["bmid"][l], [128, 128]) for l in range(6)]
    bskip = load_b("bskip", d["bskip"][:], [128, 128])
    bfin = load_b("bfin", d["bfin"][:], [128, 128])
    bsig = load_b("bsig", d["bsig"][:], [1])[0]
    nab1 = load_b("nab1", d["nab1"][:], [H])[0]
    e1b2 = load_b("e1b2", d["e1b2"][:], [H])[0]
    nab2 = load_b("nab2", d["nab2"][:], [H])[0]
    e2b2 = load_b("e2b2", d["e2b2"][:], [H])[0]
    brgb = load_b("brgb", d["brgb"][:], [3])[0]

    ACT = mybir.ActivationFunctionType

    pending = []
    for gi in range(GPC * reps):
        g = gi % GPC
        g0 = g * G
        t_in = _pre0 if gi == 0 else load_group(g)
        xt = t_in["xt"]
        kl = t_in["kl"]; ks = t_in["ks"]; ma = t_in["ma"]; mb = t_in["mb"]

        nbr = drp.tile([KNN * G], U16, tag="nbr")   # wrapped k-major image
        # addr = k*2048 + r*128 + q  <->  element m of gather list k at [r=m%16, f=m//16]
        nbr3 = nbr[:].rearrange("(k r q) -> q r k", k=KNN, r=16, q=128)
        na1 = ap2.tile([H, G], F32, tag="na1")
        ct1 = drp.tile([G, H], BF16, tag="ct1")

        def knn_mt(mt):
            ps = psp.tile([128, 2048], F32, tag="ps", name="ps")
            msl = slice(mt * 128, (mt + 1) * 128)
            for nt in range(4):
                osl = slice(nt * 512, (nt + 1) * 512)
                nc.tensor.matmul(ps[:, osl], kl[:, msl], ma[:, osl],
                                 start=True, stop=False)
            for nt in range(4):
                osl = slice(nt * 512, (nt + 1) * 512)
                nc.tensor.matmul(ps[:, osl], ks[:, msl], mb[:, osl],
                                 start=False, stop=True)
            vmax = sp.tile([128, 8], F32, tag="vmax")
            nc.vector.max(vmax[:], ps[:])
            imax = sp.tile([128, 8], U16, tag="imax")
            nc.vector.max_index(imax[:], vmax[:], ps[:])
            for k in range(KNN):
                nc.sync.dma_start(nbr3[mt * 8:(mt + 1) * 8, :, k], imax[:, 1 + k])

        def mlp_np_thunks(p):
            """MLP trunk for node-pair block p, as a list of per-layer thunks
            so PE work can be dripped between kNN row-tiles."""
            n0 = p * 1024
            sl = slice(n0, n0 + 1024)
            hsl = [slice(n0, n0 + 512), slice(n0 + 512, n0 + 1024)]
            lsl = [slice(0, 512), slice(512, 1024)]
            st = {}

            def layer_mms(ps_list, wchunks, movers):
                # ps_list: [tile for ch0, tile for ch1]; movers: list of
                # (moving AP for ns0, moving AP for ns1) per weight chunk
                for ch in range(2):
                    csl = slice(ch * 128, (ch + 1) * 128)
                    for ns in range(2):
                        for i, wk in enumerate(wchunks):
                            nc.tensor.matmul(
                                ps_list[ch][:, lsl[ns]], wk[:, csl], movers[i][ns],
                                start=(i == 0), stop=(i == len(wchunks) - 1))

            def hmov(h):
                return [(h[0][:, lsl[0]], h[0][:, lsl[1]]),
                        (h[1][:, lsl[0]], h[1][:, lsl[1]])]

            def t0():
                ps = [psm.tile([128, 1024], F32, tag="pm", name=f"ps0{ch}") for ch in range(2)]
                layer_mms(ps, [w0], [(xt[0:DXYZ, hsl[0]], xt[0:DXYZ, hsl[1]])])
                h = [hp.tile([128, 1024], F32R, tag=f"h{ch}", name=f"h{ch}") for ch in range(2)]
                for ch in range(2):
                    nc.scalar.activation(h[ch][:], ps[ch][:], ACT.Relu, bias=b0[ch][:])
                st["h"] = h

            def tl(layer, m):
                h = st["h"]
                ps = [psm.tile([128, 1024], F32, tag="pm", name=f"psl{ch}") for ch in range(2)]
                if layer == 4:
                    bk = bskip
                    layer_mms(ps, wskip,
                              [(xt[0:DXYZ, hsl[0]], xt[0:DXYZ, hsl[1]])] + hmov(h))
                else:
                    bk = bmid[m]
                    layer_mms(ps, wmid[m], hmov(h))
                hn = [hp.tile([128, 1024], F32R, tag=f"h{ch}", name=f"hn{ch}") for ch in range(2)]
                for ch in range(2):
                    nc.scalar.activation(hn[ch][:], ps[ch][:], ACT.Relu, bias=bk[ch][:])
                st["h"] = hn

            def tfin():
                movers = hmov(st["h"])
                ps = [psm.tile([128, 1024], F32, tag="pm", name=f"psf{ch}") for ch in range(2)]
                layer_mms(ps, wfin, movers)
                pss = psm.tile([1, 1024], F32, tag="pm", name="pss")
                for ns in range(2):
                    nc.tensor.matmul(pss[0:1, lsl[ns]], wsig[0][:], movers[0][ns],
                                     start=True, stop=False)
                    nc.tensor.matmul(pss[0:1, lsl[ns]], wsig[1][:], movers[1][ns],
                                     start=False, stop=True)
                feat = [fp.tile([128, 1024], F32R, tag=f"feat{ch}", name=f"feat{ch}") for ch in range(2)]
                for ch in range(2):
                    nc.scalar.activation(feat[ch][:], ps[ch][:], ACT.Identity, bias=bfin[ch][:])
                sgt = op.tile([1, 1024], F32, tag="sgt")
                nc.scalar.activation(sgt[:], pss[0:1, :], ACT.Identity, bias=bsig[:])
                nc.sync.dma_start(sig_d[:, g0 + n0:g0 + n0 + 1024], sgt[:])
                st["feat"] = feat

            def ta1c1():
                feat = st["feat"]
                fmov = hmov(feat) + [(xt[64:91, hsl[0]], xt[64:91, hsl[1]])]
                psA = psm.tile([128, 1024], F32, tag="pm", name="psA")
                for ns in range(2):
                    for i in range(3):
                        nc.tensor.matmul(psA[:, lsl[ns]], naw1[i], fmov[i][ns],
                                         start=(i == 0), stop=(i == 2))
                nc.scalar.activation(na1[:, sl], psA[:], ACT.Identity, bias=nab1[:])
                # C1 transposed: per 128-node chunk, [nodes, feats] = sum_i
                # stationary(feat-data chunk) x moving(cw1 chunk); bf16 out to
                # HBM node-major for the SWDGE gather.
                psT = psm.tile([128, 1024], F32, tag="pm", name="psT")
                for c in range(8):
                    lc = slice(c * 128, (c + 1) * 128)
                    gsl = slice(n0 + c * 128, n0 + (c + 1) * 128)
                    nc.tensor.matmul(psT[:, lc], feat[0][:, lc], cw1[0],
                                     start=True, stop=False)
                    nc.tensor.matmul(psT[:, lc], feat[1][:, lc], cw1[1],
                                     start=False, stop=False)
                    nc.tensor.matmul(psT[:, lc], xt[64:91, gsl], cw1[2],
                                     start=False, stop=True)
                ctsb = ec.tile([128, 1024], BF16, tag="ctsb")
                nc.scalar.activation(ctsb[:], psT[:], ACT.Copy)
                nc.sync.dma_start(
                    ct1[n0:n0 + 1024, :].rearrange("(c p) f -> p c f", p=128),
                    ctsb[:].rearrange("p (c f) -> p c f", c=8))

            thunks = [t0]
            m = 0
            for layer in range(1, 8):
                mm = m
                thunks.append(lambda l=layer, mi_=mm: tl(l, mi_))
                if layer != 4:
                    m += 1
            thunks += [tfin, ta1c1]
            return thunks

        # interleave knn row-tiles + MLP layer thunks + the PREVIOUS group's
        # EdgeConv thunks, so every engine's in-order queue alternates
        # between the two groups (software pipeline) and PE work is smooth.
        mlp_thunks = mlp_np_thunks(0) + mlp_np_thunks(1)
        ti = 0
        mi = 0
        nmt = len(mlp_thunks)
        for mt in range(MT):
            knn_mt(mt)
            if ti < len(pending):
                pending[ti](); ti += 1
            if mt % 8 == 7:
                while mi < (mt + 1) * nmt // MT:
                    mlp_thunks[mi](); mi += 1
                if ti < len(pending):
                    pending[ti](); ti += 1
        while mi < nmt:
            mlp_thunks[mi](); mi += 1
        while ti < len(pending):
            pending[ti](); ti += 1

        # wrapped gather indices, k-major: idxw[:, k*128+f] block for neighbor k.
        # One strided DRAM read into partitions 0:16, then replicate to all
        # 16-partition blocks (one per Q7 core) with SBUF->SBUF copies.
        idxw = idxp.tile([128, G * KNN // 16], I16, tag="idxw")
        nbr_r = nbr[:].rearrange("(k r f) -> r k f", k=KNN, r=16, f=128)
        nc.sync.dma_start(
            idxw[0:16, :].rearrange("r (k f) -> r k f", k=KNN),
            nbr_r.bitcast(I16))
        for r in range(1, 8):
            nc.sync.dma_start(idxw[16 * r:16 * r + 16, :], idxw[0:16, :])

        def make_ec_thunks(g0, idxw, na1, ct1):
            """Build the EdgeConv thunk list for this group; emitted later,
            interleaved into the NEXT group's knn/mlp emission."""
            s1 = ap1.tile([H, G], F32R, tag="s1")
            na2 = ap1.tile([H, G], F32, tag="na2")
            ct2 = drp.tile([G, H], BF16, tag="ct2")
            thunks = []

            def conv_thunks(src, nA, w2, b2, dst_of, out_cb):
                gts = {}

                def gather_half(half):
                    gt = ecg.tile([128, KNN, 1024], BF16, tag="g1")
                    gts[half] = gt
                    for k in range(KNN):
                        for h2 in range(2):
                            nc.gpsimd.dma_gather(
                                gt[:, k:k + 1, h2 * 512:(h2 + 1) * 512],
                                src[:], idxw[:, k * 128 + half * 64 + h2 * 32:
                                             k * 128 + half * 64 + h2 * 32 + 32],
                                512, 512, H, transpose=True, queue_num=nextq())

                def do_chunk(c):
                    gt, cc = gts[c // 2], c % 2
                    nsl = slice(c * 512, (c + 1) * 512)
                    nab = nA[:, nsl].unsqueeze(1).to_broadcast([H, KNN, 512])
                    msgp = ecb.tile([128, KNN, 512], F32, tag="tmx")
                    nc.vector.tensor_sub(msgp[:], gt[:, :, cc * 512:(cc + 1) * 512], nab)
                    msgr = ecb.tile([128, KNN, 512], F32R, tag="msgr")
                    nc.scalar.activation(msgr[:], msgp[:], ACT.Relu)
                    mr = msgr[:]
                    psE = psm.tile([128, 1024], F32, tag="pm", name="psE")
                    psE2 = psm.tile([128, 1024], F32, tag="pm", name="psE2")
                    nc.tensor.matmul(psE[:, 0:512], w2[:], mr[:, 0, :], start=True, stop=True)
                    nc.tensor.matmul(psE[:, 512:1024], w2[:], mr[:, 1, :], start=True, stop=True)
                    nc.tensor.matmul(psE2[:, 0:512], w2[:], mr[:, 2, :], start=True, stop=True)
                    h2 = ec.tile([128, 1024], F32, tag="h2")
                    h22 = ec.tile([128, 512], F32, tag="h22")
                    nc.scalar.activation(h2[:], psE[:], ACT.Relu, bias=b2[:])
                    nc.scalar.activation(h22[:], psE2[:, 0:512], ACT.Relu, bias=b2[:])
                    tmp = ec.tile([128, 512], F32, tag="trio")
                    nc.vector.tensor_add(tmp[:], h2[:, 0:512], h2[:, 512:1024])
                    dst = dst_of(c)
                    nc.vector.tensor_add(dst, tmp[:], h22[:])
                    out_cb(c, dst)

                for half in range(2):
                    thunks.append(lambda h=half: gather_half(h))
                    thunks.append(lambda c=half * 2: do_chunk(c))
                    thunks.append(lambda c=half * 2 + 1: do_chunk(c))

            # ---- EdgeConv 1 ----
            conv_thunks(ct1, na1, e1w2, e1b2,
                        lambda c: s1[:, c * 512:(c + 1) * 512], lambda c, dstap: None)

            # ---- A2 / C2 ----
            def a2c2(p):
                lsl = [slice(p * 1024, p * 1024 + 512), slice(p * 1024 + 512, p * 1024 + 1024)]
                s1r = s1[:]
                psA = psm.tile([128, 1024], F32, tag="pm", name="psA2")
                for ns in range(2):
                    nc.tensor.matmul(psA[:, ns * 512:(ns + 1) * 512], na2w[:], s1r[:, lsl[ns]],
                                     start=True, stop=True)
                nc.scalar.activation(na2[:, p * 1024:(p + 1) * 1024], psA[:], ACT.Identity, bias=nab2[:])
                psT = psm.tile([128, 1024], F32, tag="pm", name="psT2")
                for c in range(8):
                    lc = slice(c * 128, (c + 1) * 128)
                    gsl = slice(p * 1024 + c * 128, p * 1024 + (c + 1) * 128)
                    nc.tensor.matmul(psT[:, lc], s1r[:, gsl], c2w[:],
                                     start=True, stop=True)
                ctsb = ec.tile([128, 1024], BF16, tag="ctsb")
                nc.scalar.activation(ctsb[:], psT[:], ACT.Copy)
                nc.sync.dma_start(
                    ct2[p * 1024:(p + 1) * 1024, :].rearrange("(c p) f -> p c f", p=128),
                    ctsb[:].rearrange("p (c f) -> p c f", c=8))

            thunks.append(lambda: a2c2(0))
            thunks.append(lambda: a2c2(1))

            # ---- EdgeConv 2 ----
            s2t = ec.tile([128, 512], F32R, tag="s2final")

            def ec2_out(c, dstap):
                psR = psm.tile([3, 512], F32, tag="pm", name="psR")
                nc.tensor.matmul(psR[0:3, 0:512], wrgb[:], dstap,
                                 start=True, stop=True)
                rgt = op.tile([3, 512], F32, tag="rgt")
                nc.scalar.activation(rgt[:], psR[0:3, 0:512], ACT.Sigmoid, bias=brgb[:])
                nc.sync.dma_start(rgb_d[:, g0 + c * 512:g0 + (c + 1) * 512], rgt[:])

            conv_thunks(ct2, na2, e2w2, e2b2, lambda c: s2t[:], ec2_out)
            return thunks

        pending = make_ec_thunks(g0, idxw, na1, ct1)

    for t in pending:
        t()

    for p in reversed(ctxs):
        p.__exit__(None, None, None)


def _core_groups():
    cg = []
    for c in range(N_CORES):
        if c < 4:
            gs = [3 * c, 3 * c + 1, 3 * c + 2]
        else:
            g0 = 12 + 2 * (c - 4)
            gs = [g0, g0 + 1, g0]  # 3rd slot = dummy repeat
        cg.append(gs)
    return cg


def _prep(inputs):
    x = np.asarray(inputs["x"], dtype=np.float32)
    batch_ids = np.asarray(inputs["batch_ids"])
    perm = np.argsort(batch_ids, kind="stable")
    xs = np.ascontiguousarray(x[perm])

    xyz = xs[:, :DXYZ]
    sq = (xyz * xyz).sum(1, dtype=np.float32)

    w = {k: np.asarray(inputs[k], dtype=np.float32) for k in inputs if k not in ("x", "batch_ids")}
    e1 = w["e1_w1"]
    naw1 = np.ascontiguousarray(-(e1[:W + DDIR] - e1[W + DDIR:]))
    cw1 = np.ascontiguousarray(e1[W + DDIR:])
    e2 = w["e2_w1"]
    na2w = np.ascontiguousarray(-(e2[:H] - e2[H:]) / 3.0)
    c2w = np.ascontiguousarray(e2[H:] / 3.0)

    shared = {
        "w0": w["w0"], "b0": w["b0"].reshape(W, 1),
        "wmid": w["w_mid"], "bmid": w["b_mid"].reshape(6, W, 1),
        "wskip": w["w_skip"], "bskip": w["b_skip"].reshape(W, 1),
        "wfin": w["w_final"], "bfin": w["b_final"].reshape(W, 1),
        "wsig": w["w_sigma"], "bsig": w["b_sigma"].reshape(1, 1),
        "naw1": naw1, "cw1": cw1, "nab1": -w["e1_b1"].reshape(H, 1),
        "e1w2": w["e1_w2"], "e1b2": w["e1_b2"].reshape(H, 1),
        "na2w": na2w, "c2w": c2w, "nab2": -w["e2_b1"].reshape(H, 1),
        "e2w2": w["e2_w2"], "e2b2": w["e2_b2"].reshape(H, 1),
        "wrgb": np.ascontiguousarray(w["w_rgb"] / 3.0), "brgb": w["b_rgb"].reshape(3, 1),
    }
    shared = {k: np.ascontiguousarray(v, dtype=np.float32) for k, v in shared.items()}

    in_maps = []
    for gs in _core_groups():
        rows = np.concatenate([np.arange(g * G, (g + 1) * G) for g in gs])
        xc = xs[rows]
        xyzT = np.ascontiguousarray(xc[:, :DXYZ].T)   # [63, NODES] f32
        xt = np.empty((91, NODES), np.float32)
        xt[0:DXYZ] = xyzT
        xt[DXYZ] = 1.0
        xt[DXYZ + 1:] = xc[:, DXYZ:].T

        a = xyzT.astype(NPBF)
        bb = (xyzT - a.astype(np.float32)).astype(NPBF)
        nh = -0.5 * sq[rows]
        s1 = nh.astype(NPBF)
        s2 = (nh - s1.astype(np.float32)).astype(NPBF)
        ones2 = np.ones((3, NODES), NPBF)
        zeros1 = np.zeros((1, NODES), NPBF)

        m = dict(shared)
        m["xt"] = np.ascontiguousarray(xt)
        m["kl"] = np.ascontiguousarray(np.concatenate([a, bb], 0))
        m["ks"] = np.ascontiguousarray(np.concatenate([a, ones2], 0))
        m["ma"] = np.ascontiguousarray(np.concatenate([a, a], 0))
        m["mb"] = np.ascontiguousarray(np.concatenate([bb, s1[None], s2[None], zeros1], 0))
        in_maps.append(m)
    return in_maps, perm


def _assemble(results, perm):
    out_sorted = np.empty((B, 4), np.float32)
    for c, gs in enumerate(_core_groups()):
        r = results[c]
        for slot, g in enumerate(gs):
            if c >= 4 and slot == 2:
                continue  # dummy
            sl = slice(slot * G, (slot + 1) * G)
            out_sorted[g * G:(g + 1) * G, 0:3] = r["rgb"][:, sl].T
            out_sorted[g * G:(g + 1) * G, 3] = r["sig"][0, sl]
    out = np.empty((B, 4), np.float32)
    out[perm] = out_sorted
    return out


def get_nc(reps=1):
    key = f"nc{reps}"
    if key not in _STATE:
        _STATE[key] = _build_nc(reps)
    return _STATE[key]


def kernel(**inputs) -> np.ndarray:
    nc = get_nc()
    in_maps, perm = _prep(inputs)
    res = bass_utils.run_bass_kernel_spmd(nc, in_maps, core_ids=list(range(N_CORES)))
    return _assemble(res.results, perm)



# revision 26
# speedup vs baseline: 3.6618x; 1.3283x over previous
"""Trainium2 Bass kernel for nn_NeRFGraph (gnn_message_passing).

Strategy (sharding_hint): nodes are sharded across 8 cores aligned to whole
knn batch groups. 20 groups of 2048 nodes -> cores 0-3 take 3 groups,
cores 4-7 take 2 real groups + 1 dummy (SPMD needs uniform shapes; dummy
output is dropped on the host). MLP weights are replicated (data parallel).

Per-core pipeline, per group g (layouts are [features(partitions), nodes(free)]):
  1. kNN via ONE bf16 matmul per [128,512] score tile:
     score = a_i.a_j + s1_j + s2_j with a = bf16(x), s1+s2 = hi/lo bf16 split
     of -|x|^2/2. The dropped hi/lo cross terms flip ~500/40960 neighbor
     sets (measured end-to-end 2.2e-3 rel; tolerance 2e-2). Self always wins
     top-1, so neighbors = entries 1..3 of the DVE max8/max_index scan.
     Emission is software-pipelined: tile t's scan is emitted before tile
     t+1's matmuls so the PSUM refill sits ahead of dripped filler work in
     the in-order PE queue.
  2. MLP (8 layers + skip at 4) in float32r, node-pair blocks of 1024;
     thunks dripped evenly between kNN row tiles (PE fills DVE-scan gaps).
  3. EdgeConv x2, factorized: msg_ij = relu(A_i + C_j). A tables on-chip
     (f32); C tables computed TRANSPOSED (per 128-node chunk, stationary =
     feat data, moving = C-weights; all-bf16 matmuls) -> bf16 node-major
     [2048,128] tables staged to HBM, then gathered per edge with SWDGE
     dma_gather (transpose=True, elem_size=128, 512-idx chunks on 4
     rotating queues; queue = emission-index %% 4 to match Tile's DMASW sem
     rotation; chunks of 1024+ idxs overflow the 1024-entry descriptor ring
     and hang the device). This replaced gpsimd ap_gather, which at
     ~30us/1024-idx call was 87%% of runtime. W2 matmul + mean over K=3
     folded into next layer's weights (host prescale by 1/3). EdgeConv of
     group g is emitted interleaved into group g+1's kNN/MLP emission.
     NOTE (HW erratum): f32r matmuls with moving size < 256 run in a
     replicated multi-pass mode whose accumulation corrupts across PE
     p-state transitions (CoreSim is clean; HW produced ~3x-accumulated
     garbage in exactly the chunks before the 3us ramp boundary). ALL
     narrow-N table matmuls are bf16 for this reason; keep f32r matmuls at
     moving size >= 512. Also: bf16 DVE msg ops and finer drip measured
     SLOWER on HW than f32 despite sim predictions - keep msg path f32 and
     validate every scheduling change by on-device A/B (T(9) vs T(1)
     differencing), not by the simulator.
  4. rgb = sigmoid(S2 @ w_rgb/3 + b_rgb), sigma from the MLP trunk.
"""

import numpy as np
import ml_dtypes

import concourse.bass as bass
import concourse.tile as tile
from concourse import bacc, mybir, library_config
import concourse.bass_utils as bass_utils

F32 = mybir.dt.float32
F32R = mybir.dt.float32r
BF16 = mybir.dt.bfloat16
U16 = mybir.dt.uint16
I16 = mybir.dt.int16
NPBF = ml_dtypes.bfloat16

# problem constants (hardcoded per contract)
B = 40960
NG = 20
DXYZ = 63
DDIR = 27
W = 256
H = 128  # W // 2
KNN = 3

N_CORES = 8
GPC = 3                      # groups per core (SPMD-uniform)
G = B // NG                  # 2048 nodes per group
NODES = GPC * G              # 6144 nodes per core
MT = G // 128                # row tiles of 128 per group (knn)
NP = G // 1024               # node pairs of 1024 per group (mlp)

_STATE: dict = {}


def _build_nc(reps=1):
    nc = bacc.Bacc(
        "TRN2",
        target_bir_lowering=False,
        debug=False,
        enable_asserts=False,
        num_devices=N_CORES,
        num_swdge_queues=4,
    )
    d = {}

    def inp(name, shape, dt=F32):
        d[name] = nc.dram_tensor(name, list(shape), dt, kind="ExternalInput").ap()

    inp("xt", (91, NODES), F32R)    # rows 0-62 xyz, 63 ones, 64-90 dir (f32 bits)
    inp("kl", (126, NODES), BF16)   # [a; b]      (unused pad: layout/timing)
    inp("ks", (66, NODES), BF16)    # [a; 1; 1; 1] stationary (pad to 66)
    inp("ma", (126, NODES), BF16)   # [a; a]      (unused pad: layout/timing)
    inp("mb", (66, NODES), BF16)    # [a; s1; s2; 0] moving
    inp("xdb", (DDIR, NODES), BF16)  # dir rows (bf16 copy for table matmuls)
    inp("w0", (DXYZ, W)); inp("b0", (W, 1))
    inp("wmid", (6, W, W))          # [layer, in, out]
    inp("bmid", (6, W, 1))
    inp("wskip", (DXYZ + W, W)); inp("bskip", (W, 1))
    inp("wfin", (W, W)); inp("bfin", (W, 1))
    inp("wsig", (W, 1)); inp("bsig", (1, 1))
    inp("naw1", (W + DDIR, H)); inp("cw1", (W + DDIR, H)); inp("nab1", (H, 1))
    inp("e1w2", (H, H)); inp("e1b2", (H, 1))
    inp("na2w", (H, H)); inp("c2w", (H, H)); inp("nab2", (H, 1))
    inp("e2w2", (H, H)); inp("e2b2", (H, 1))
    inp("wrgb", (H, 3)); inp("brgb", (3, 1))

    rgb_d = nc.dram_tensor("rgb", [3, NODES], F32, kind="ExternalOutput").ap()

    sig_d = nc.dram_tensor("sig", [1, NODES], F32, kind="ExternalOutput").ap()

    with tile.TileContext(nc) as tc:
        _body(tc, d, rgb_d, sig_d, reps=reps)
    nc.compile()
    return nc


def _body(tc, d, rgb_d, sig_d, reps=1):
    nc = tc.nc
    ctxs = []

    def pool(name, bufs, space="SBUF"):
        p = tc.tile_pool(name=name, bufs=bufs, space=space)
        ctxs.append(p)
        return p.__enter__()

    wstage = pool("wstage", bufs=1)       # f32 staging for weight rounding
    wp = pool("wp", bufs=1)               # persistent rounded weights / biases
    xp = pool("xp", bufs=2)               # per-group xt (f32r)
    xk = pool("xk", bufs=1)               # knn bf16 inputs
    ap2 = pool("ap2", bufs=2)             # per-group na1/c1 gather tables
    ap1 = pool("ap1", bufs=1)             # per-group s1/na2/c2 tables
    ec = pool("ec", bufs=2)               # edge-conv small chunk tiles
    ecb = pool("ecb", bufs=2)             # edge-conv big msg tiles (Pool-only)
    ecg = pool("ecg", bufs=2)             # full-group gather outputs [128,3,G]
    hp = pool("hp", bufs=2)               # MLP hidden tiles [128,1024]
    fp = pool("fp", bufs=1)               # feat tiles [128,1024]
    sp = pool("sp", bufs=6)               # small tiles (vmax/imax)
    op = pool("op", bufs=1)               # output staging
    idxp = pool("idxp", bufs=2)
    psp = pool("psp", bufs=1, space="PSUM")    # knn scores [128,2048]
    psm = pool("psm", bufs=2, space="PSUM")    # everything else [128,1024]
    drp = pool("drp", bufs=2, space="DRAM")

    nc.gpsimd.load_library(library_config.mlp)
    # SWDGE queue assignment must match Tile's DMASW sem rotation (8 sems,
    # round-robin): queue = (emission index of Pool-engine DMA insts) % 4.
    swq = [0]

    def nextq():
        q = swq[0] % 4
        swq[0] += 1
        return q

    # ---- per-group input loads (group 0 emitted BEFORE the weight loads so
    # its DMAs are first in the HWDGE queues and kNN can start immediately) ----
    def load_group(g):
        g0 = g * G
        t = {}
        t["xt"] = xp.tile([91, G], F32R, tag="xt", name="xt")
        nc.sync.dma_start(t["xt"][:], d["xt"][:, g0:g0 + G])
        for nm, rows in (("kl", 126), ("ks", 66), ("ma", 126), ("mb", 66),
                         ("xdb", DDIR)):
            t[nm] = xk.tile([rows, G], BF16, tag=nm, name=nm)
            nc.sync.dma_start(t[nm][:], d[nm][:, g0:g0 + G])
        return t

    _pre0 = load_group(0)

    # ---- load + round weights to f32r (one-time) ----
    def load_chunks(src_ap, rows, cols, tag, part_off=0, dt=F32R):
        """src_ap: DRAM AP [R, cols]; returns list of rounded chunk tile APs.
        part_off: place the LAST chunk at this base partition (32-aligned)."""
        out = []
        r0 = 0
        for i, r in enumerate(rows):
            last = i == len(rows) - 1
            if last and part_off:
                st = wstage.tile([part_off + r, cols], F32, tag="wstage_p", name="stp")
                nc.sync.dma_start(st[part_off:part_off + r, :], src_ap[r0:r0 + r, :])
                wt = wp.tile([part_off + r, cols], dt, tag=f"{tag}_{i}", name="wtp")
                nc.scalar.activation(wt[part_off:part_off + r, :],
                                     st[part_off:part_off + r, :],
                                     mybir.ActivationFunctionType.Identity)
                out.append(wt[part_off:part_off + r, :])
            else:
                st = wstage.tile([r, cols], F32, tag="wstage", name="st")
                nc.sync.dma_start(st[:], src_ap[r0:r0 + r, :])
                wt = wp.tile([r, cols], dt, tag=f"{tag}_{i}", name="wt")
                nc.scalar.activation(wt[:], st[:], mybir.ActivationFunctionType.Identity)
                out.append(wt[:])
            r0 += r
        return out

    def load_b(name, src_ap, rows):
        out = []
        r0 = 0
        for i, r in enumerate(rows):
            bt = wp.tile([r, 1], F32, tag=f"{name}_{i}", name="bt")
            nc.sync.dma_start(bt[:], src_ap[r0:r0 + r, :])
            out.append(bt)
            r0 += r
        return out

    w0 = load_chunks(d["w0"][:], [DXYZ], W, "w0")[0]
    wmid = [load_chunks(d["wmid"][l], [128, 128], W, f"wmid{l}") for l in range(6)]
    wskip = load_chunks(d["wskip"][:], [DXYZ, 128, 128], W, "wskip")
    wfin = load_chunks(d["wfin"][:], [128, 128], W, "wfin")
    wsig = load_chunks(d["wsig"][:], [128, 128], 1, "wsig")
    naw1 = load_chunks(d["naw1"][:], [128, 128, DDIR], H, "naw1", dt=BF16)
    cw1 = load_chunks(d["cw1"][:], [128, 128, DDIR], H, "cw1", dt=BF16)
    e1w2 = load_chunks(d["e1w2"][:], [H], H, "e1w2")[0]
    na2w = load_chunks(d["na2w"][:], [H], H, "na2w", dt=BF16)[0]
    c2w = load_chunks(d["c2w"][:], [H], H, "c2w", dt=BF16)[0]
    e2w2 = load_chunks(d["e2w2"][:], [H], H, "e2w2")[0]
    wrgb = load_chunks(d["wrgb"][:], [H], 3, "wrgb")[0]

    b0 = load_b("b0", d["b0"][:], [128, 128])
    bmid = [load_b(f"bmid{l}", d["bmid"][l], [128, 128]) for l in range(6)]
    bskip = load_b("bskip", d["bskip"][:], [128, 128])
    bfin = load_b("bfin", d["bfin"][:], [128, 128])
    bsig = load_b("bsig", d["bsig"][:], [1])[0]
    nab1 = load_b("nab1", d["nab1"][:], [H])[0]
    e1b2 = load_b("e1b2", d["e1b2"][:], [H])[0]
    nab2 = load_b("nab2", d["nab2"][:], [H])[0]
    e2b2 = load_b("e2b2", d["e2b2"][:], [H])[0]
    brgb = load_b("brgb", d["brgb"][:], [3])[0]

    ACT = mybir.ActivationFunctionType

    pending = []
    for gi in range(GPC * reps):
        g = gi % GPC
        g0 = g * G
        t_in = _pre0 if gi == 0 else load_group(g)
        xt = t_in["xt"]
        ks = t_in["ks"]; mb = t_in["mb"]; xdb = t_in["xdb"]

        nbr = drp.tile([KNN * G], U16, tag="nbr")   # wrapped k-major image
        # addr = k*2048 + r*128 + q  <->  element m of gather list k at [r=m%16, f=m//16]
        nbr3 = nbr[:].rearrange("(k r q) -> q r k", k=KNN, r=16, q=128)
        na1 = ap2.tile([H, G], F32, tag="na1")
        ct1 = drp.tile([G, H], BF16, tag="ct1")

        def knn_mm(mt):
            ps = psp.tile([128, 2048], F32, tag="ps", name="ps")
            msl = slice(mt * 128, (mt + 1) * 128)
            for nt in range(4):
                osl = slice(nt * 512, (nt + 1) * 512)
                nc.tensor.matmul(ps[:, osl], ks[:, msl], mb[:, osl],
                                 start=True, stop=True)
            return ps

        def knn_scan(mt, ps):
            vmax = sp.tile([128, 8], F32, tag="vmax")
            nc.vector.max(vmax[:], ps[:])
            imax = sp.tile([128, 8], U16, tag="imax")
            nc.vector.max_index(imax[:], vmax[:], ps[:])
            for k in range(KNN):
                nc.sync.dma_start(nbr3[mt * 8:(mt + 1) * 8, :, k], imax[:, 1 + k])

        def mlp_np_thunks(p):
            """MLP trunk for node-pair block p, as a list of per-layer thunks
            so PE work can be dripped between kNN row-tiles."""
            n0 = p * 1024
            sl = slice(n0, n0 + 1024)
            hsl = [slice(n0, n0 + 512), slice(n0 + 512, n0 + 1024)]
            lsl = [slice(0, 512), slice(512, 1024)]
            st = {}

            def layer_mms(ps_list, wchunks, movers):
                # ps_list: [tile for ch0, tile for ch1]; movers: list of
                # (moving AP for ns0, moving AP for ns1) per weight chunk
                for ch in range(2):
                    csl = slice(ch * 128, (ch + 1) * 128)
                    for ns in range(2):
                        for i, wk in enumerate(wchunks):
                            nc.tensor.matmul(
                                ps_list[ch][:, lsl[ns]], wk[:, csl], movers[i][ns],
                                start=(i == 0), stop=(i == len(wchunks) - 1))

            def hmov(h):
                return [(h[0][:, lsl[0]], h[0][:, lsl[1]]),
                        (h[1][:, lsl[0]], h[1][:, lsl[1]])]

            def t0():
                ps = [psm.tile([128, 1024], F32, tag="pm", name=f"ps0{ch}") for ch in range(2)]
                layer_mms(ps, [w0], [(xt[0:DXYZ, hsl[0]], xt[0:DXYZ, hsl[1]])])
                h = [hp.tile([128, 1024], F32R, tag=f"h{ch}", name=f"h{ch}") for ch in range(2)]
                for ch in range(2):
                    nc.scalar.activation(h[ch][:], ps[ch][:], ACT.Relu, bias=b0[ch][:])
                st["h"] = h

            def tl(layer, m):
                h = st["h"]
                ps = [psm.tile([128, 1024], F32, tag="pm", name=f"psl{ch}") for ch in range(2)]
                if layer == 4:
                    bk = bskip
                    layer_mms(ps, wskip,
                              [(xt[0:DXYZ, hsl[0]], xt[0:DXYZ, hsl[1]])] + hmov(h))
                else:
                    bk = bmid[m]
                    layer_mms(ps, wmid[m], hmov(h))
                hn = [hp.tile([128, 1024], F32R, tag=f"h{ch}", name=f"hn{ch}") for ch in range(2)]
                for ch in range(2):
                    nc.scalar.activation(hn[ch][:], ps[ch][:], ACT.Relu, bias=bk[ch][:])
                st["h"] = hn

            def tfin():
                movers = hmov(st["h"])
                ps = [psm.tile([128, 1024], F32, tag="pm", name=f"psf{ch}") for ch in range(2)]
                layer_mms(ps, wfin, movers)
                pss = psm.tile([1, 1024], F32, tag="pm", name="pss")
                for ns in range(2):
                    nc.tensor.matmul(pss[0:1, lsl[ns]], wsig[0][:], movers[0][ns],
                                     start=True, stop=False)
                    nc.tensor.matmul(pss[0:1, lsl[ns]], wsig[1][:], movers[1][ns],
                                     start=False, stop=True)
                feat = [fp.tile([128, 1024], BF16, tag=f"feat{ch}", name=f"feat{ch}") for ch in range(2)]
                for ch in range(2):
                    nc.scalar.activation(feat[ch][:], ps[ch][:], ACT.Identity, bias=bfin[ch][:])
                sgt = op.tile([1, 1024], F32, tag="sgt")
                nc.scalar.activation(sgt[:], pss[0:1, :], ACT.Identity, bias=bsig[:])
                nc.sync.dma_start(sig_d[:, g0 + n0:g0 + n0 + 1024], sgt[:])
                st["feat"] = feat

            def ta1c1():
                feat = st["feat"]
                fmov = hmov(feat) + [(xdb[:, hsl[0]], xdb[:, hsl[1]])]
                psA = psm.tile([128, 1024], F32, tag="pm", name="psA")
                for ns in range(2):
                    for i in range(3):
                        nc.tensor.matmul(psA[:, lsl[ns]], naw1[i], fmov[i][ns],
                                         start=(i == 0), stop=(i == 2))
                nc.scalar.activation(na1[:, sl], psA[:], ACT.Identity, bias=nab1[:])
                # C1 transposed: per 128-node chunk, [nodes, feats] = sum_i
                # stationary(feat-data chunk) x moving(cw1 chunk); bf16 out to
                # HBM node-major for the SWDGE gather.
                psT = psm.tile([128, 1024], F32, tag="pm", name="psT")
                for c in range(8):
                    lc = slice(c * 128, (c + 1) * 128)
                    gsl = slice(n0 + c * 128, n0 + (c + 1) * 128)
                    nc.tensor.matmul(psT[:, lc], feat[0][:, lc], cw1[0],
                                     start=True, stop=False)
                    nc.tensor.matmul(psT[:, lc], feat[1][:, lc], cw1[1],
                                     start=False, stop=False)
                    nc.tensor.matmul(psT[:, lc], xdb[:, gsl], cw1[2],
                                     start=False, stop=True)
                ctsb = ec.tile([128, 1024], BF16, tag="ctsb")
                nc.scalar.activation(ctsb[:], psT[:], ACT.Copy)
                nc.sync.dma_start(
                    ct1[n0:n0 + 1024, :].rearrange("(c p) f -> p c f", p=128),
                    ctsb[:].rearrange("p (c f) -> p c f", c=8))

            thunks = [t0]
            m = 0
            for layer in range(1, 8):
                mm = m
                thunks.append(lambda l=layer, mi_=mm: tl(l, mi_))
                if layer != 4:
                    m += 1
            thunks += [tfin, ta1c1]
            return thunks

        # interleave knn row-tiles + MLP layer thunks + the PREVIOUS group's
        # EdgeConv thunks, so every engine's in-order queue alternates
        # between the two groups (software pipeline) and PE work is smooth.
        mlp_thunks = mlp_np_thunks(0) + mlp_np_thunks(1)
        ti = 0
        mi = 0
        nmt = len(mlp_thunks)
        ps_cur = knn_mm(0)
        for mt in range(MT):
            knn_scan(mt, ps_cur)
            if mt + 1 < MT:
                ps_cur = knn_mm(mt + 1)
            if ti < len(pending):
                pending[ti](); ti += 1
            while mi < (mt + 1) * nmt // MT:
                mlp_thunks[mi](); mi += 1
            if mt % 8 == 7 and ti < len(pending):
                pending[ti](); ti += 1
        while mi < nmt:
            mlp_thunks[mi](); mi += 1
        while ti < len(pending):
            pending[ti](); ti += 1

        # wrapped gather indices, k-major: idxw[:, k*128+f] block for neighbor k.
        # One strided DRAM read into partitions 0:16, then replicate to all
        # 16-partition blocks (one per Q7 core) with SBUF->SBUF copies.
        idxw = idxp.tile([128, G * KNN // 16], I16, tag="idxw")
        nbr_r = nbr[:].rearrange("(k r f) -> r k f", k=KNN, r=16, f=128)
        nc.sync.dma_start(
            idxw[0:16, :].rearrange("r (k f) -> r k f", k=KNN),
            nbr_r.bitcast(I16))
        for r in range(1, 8):
            nc.sync.dma_start(idxw[16 * r:16 * r + 16, :], idxw[0:16, :])

        def make_ec_thunks(g0, idxw, na1, ct1):
            """Build the EdgeConv thunk list for this group; emitted later,
            interleaved into the NEXT group's knn/mlp emission."""
            s1 = ap1.tile([H, G], BF16, tag="s1")
            na2 = ap1.tile([H, G], F32, tag="na2")
            ct2 = drp.tile([G, H], BF16, tag="ct2")
            thunks = []

            def conv_thunks(src, nA, w2, b2, dst_of, out_cb):
                gts = {}

                def gather_half(half):
                    gt = ecg.tile([128, KNN, 1024], BF16, tag="g1")
                    gts[half] = gt
                    for k in range(KNN):
                        for h2 in range(2):
                            nc.gpsimd.dma_gather(
                                gt[:, k:k + 1, h2 * 512:(h2 + 1) * 512],
                                src[:], idxw[:, k * 128 + half * 64 + h2 * 32:
                                             k * 128 + half * 64 + h2 * 32 + 32],
                                512, 512, H, transpose=True, queue_num=nextq())

                def do_chunk(c):
                    gt, cc = gts[c // 2], c % 2
                    nsl = slice(c * 512, (c + 1) * 512)
                    nab = nA[:, nsl].unsqueeze(1).to_broadcast([H, KNN, 512])
                    msgp = ecb.tile([128, KNN, 512], F32, tag="tmx")
                    nc.vector.tensor_sub(msgp[:], gt[:, :, cc * 512:(cc + 1) * 512], nab)
                    msgr = ecb.tile([128, KNN, 512], F32R, tag="msgr")
                    nc.scalar.activation(msgr[:], msgp[:], ACT.Relu)
                    mr = msgr[:]
                    psE = psm.tile([128, 1024], F32, tag="pm", name="psE")
                    psE2 = psm.tile([128, 1024], F32, tag="pm", name="psE2")
                    nc.tensor.matmul(psE[:, 0:512], w2[:], mr[:, 0, :], start=True, stop=True)
                    nc.tensor.matmul(psE[:, 512:1024], w2[:], mr[:, 1, :], start=True, stop=True)
                    nc.tensor.matmul(psE2[:, 0:512], w2[:], mr[:, 2, :], start=True, stop=True)
                    h2 = ec.tile([128, 1024], F32, tag="h2")
                    h22 = ec.tile([128, 512], F32, tag="h22")
                    nc.scalar.activation(h2[:], psE[:], ACT.Relu, bias=b2[:])
                    nc.scalar.activation(h22[:], psE2[:, 0:512], ACT.Relu, bias=b2[:])
                    tmp = ec.tile([128, 512], F32, tag="trio")
                    nc.vector.tensor_add(tmp[:], h2[:, 0:512], h2[:, 512:1024])
                    dst = dst_of(c)
                    nc.vector.tensor_add(dst, tmp[:], h22[:])
                    out_cb(c, dst)

                for half in range(2):
                    thunks.append(lambda h=half: gather_half(h))
                    thunks.append(lambda c=half * 2: do_chunk(c))
                    thunks.append(lambda c=half * 2 + 1: do_chunk(c))

            # ---- EdgeConv 1 ----
            conv_thunks(ct1, na1, e1w2, e1b2,
                        lambda c: s1[:, c * 512:(c + 1) * 512], lambda c, dstap: None)

            # ---- A2 / C2 ----
            def a2c2(p):
                lsl = [slice(p * 1024, p * 1024 + 512), slice(p * 1024 + 512, p * 1024 + 1024)]
                s1r = s1[:]
                psA = psm.tile([128, 1024], F32, tag="pm", name="psA2")
                for ns in range(2):
                    nc.tensor.matmul(psA[:, ns * 512:(ns + 1) * 512], na2w[:], s1r[:, lsl[ns]],
                                     start=True, stop=True)
                nc.scalar.activation(na2[:, p * 1024:(p + 1) * 1024], psA[:], ACT.Identity, bias=nab2[:])
                psT = psm.tile([128, 1024], F32, tag="pm", name="psT2")
                for c in range(8):
                    lc = slice(c * 128, (c + 1) * 128)
                    gsl = slice(p * 1024 + c * 128, p * 1024 + (c + 1) * 128)
                    nc.tensor.matmul(psT[:, lc], s1r[:, gsl], c2w[:],
                                     start=True, stop=True)
                ctsb = ec.tile([128, 1024], BF16, tag="ctsb")
                nc.scalar.activation(ctsb[:], psT[:], ACT.Copy)
                nc.sync.dma_start(
                    ct2[p * 1024:(p + 1) * 1024, :].rearrange("(c p) f -> p c f", p=128),
                    ctsb[:].rearrange("p (c f) -> p c f", c=8))

            thunks.append(lambda: a2c2(0))
            thunks.append(lambda: a2c2(1))

            # ---- EdgeConv 2 ----
            s2t = ec.tile([128, 512], F32R, tag="s2final")

            def ec2_out(c, dstap):
                psR = psm.tile([3, 512], F32, tag="pm", name="psR")
                nc.tensor.matmul(psR[0:3, 0:512], wrgb[:], dstap,
                                 start=True, stop=True)
                rgt = op.tile([3, 512], F32, tag="rgt")
                nc.scalar.activation(rgt[:], psR[0:3, 0:512], ACT.Sigmoid, bias=brgb[:])
                nc.sync.dma_start(rgb_d[:, g0 + c * 512:g0 + (c + 1) * 512], rgt[:])

            conv_thunks(ct2, na2, e2w2, e2b2, lambda c: s2t[:], ec2_out)
            return thunks

        pending = make_ec_thunks(g0, idxw, na1, ct1)

    for t in pending:
        t()

    for p in reversed(ctxs):
        p.__exit__(None, None, None)


def _core_groups():
    cg = []
    for c in range(N_CORES):
        if c < 4:
            gs = [3 * c, 3 * c + 1, 3 * c + 2]
        else:
            g0 = 12 + 2 * (c - 4)
            gs = [g0, g0 + 1, g0]  # 3rd slot = dummy repeat
        cg.append(gs)
    return cg


def _prep(inputs):
    x = np.asarray(inputs["x"], dtype=np.float32)
    batch_ids = np.asarray(inputs["batch_ids"])
    perm = np.argsort(batch_ids, kind="stable")
    xs = np.ascontiguousarray(x[perm])

    xyz = xs[:, :DXYZ]
    sq = (xyz * xyz).sum(1, dtype=np.float32)

    w = {k: np.asarray(inputs[k], dtype=np.float32) for k in inputs if k not in ("x", "batch_ids")}
    e1 = w["e1_w1"]
    naw1 = np.ascontiguousarray(-(e1[:W + DDIR] - e1[W + DDIR:]))
    cw1 = np.ascontiguousarray(e1[W + DDIR:])
    e2 = w["e2_w1"]
    na2w = np.ascontiguousarray(-(e2[:H] - e2[H:]) / 3.0)
    c2w = np.ascontiguousarray(e2[H:] / 3.0)

    shared = {
        "w0": w["w0"], "b0": w["b0"].reshape(W, 1),
        "wmid": w["w_mid"], "bmid": w["b_mid"].reshape(6, W, 1),
        "wskip": w["w_skip"], "bskip": w["b_skip"].reshape(W, 1),
        "wfin": w["w_final"], "bfin": w["b_final"].reshape(W, 1),
        "wsig": w["w_sigma"], "bsig": w["b_sigma"].reshape(1, 1),
        "naw1": naw1, "cw1": cw1, "nab1": -w["e1_b1"].reshape(H, 1),
        "e1w2": w["e1_w2"], "e1b2": w["e1_b2"].reshape(H, 1),
        "na2w": na2w, "c2w": c2w, "nab2": -w["e2_b1"].reshape(H, 1),
        "e2w2": w["e2_w2"], "e2b2": w["e2_b2"].reshape(H, 1),
        "wrgb": np.ascontiguousarray(w["w_rgb"] / 3.0), "brgb": w["b_rgb"].reshape(3, 1),
    }
    shared = {k: np.ascontiguousarray(v, dtype=np.float32) for k, v in shared.items()}

    in_maps = []
    for gs in _core_groups():
        rows = np.concatenate([np.arange(g * G, (g + 1) * G) for g in gs])
        xc = xs[rows]
        xyzT = np.ascontiguousarray(xc[:, :DXYZ].T)   # [63, NODES] f32
        xt = np.empty((91, NODES), np.float32)
        xt[0:DXYZ] = xyzT
        xt[DXYZ] = 1.0
        xt[DXYZ + 1:] = xc[:, DXYZ:].T

        a = xyzT.astype(NPBF)
        nh = -0.5 * sq[rows]
        s1 = nh.astype(NPBF)
        s2 = (nh - s1.astype(np.float32)).astype(NPBF)
        ones2 = np.ones((3, NODES), NPBF)
        zeros1 = np.zeros((1, NODES), NPBF)

        bb = (xyzT - a.astype(np.float32)).astype(NPBF)
        m = dict(shared)
        m["xt"] = np.ascontiguousarray(xt)
        m["xdb"] = np.ascontiguousarray(xc[:, DXYZ:].T.astype(NPBF))
        m["kl"] = np.ascontiguousarray(np.concatenate([a, bb], 0))
        m["ks"] = np.ascontiguousarray(np.concatenate([a, ones2], 0))
        m["ma"] = np.ascontiguousarray(np.concatenate([a, a], 0))
        m["mb"] = np.ascontiguousarray(np.concatenate([a, s1[None], s2[None], zeros1], 0))
        in_maps.append(m)
    return in_maps, perm


def _assemble(results, perm):
    out_sorted = np.empty((B, 4), np.float32)
    for c, gs in enumerate(_core_groups()):
        r = results[c]
        for slot, g in enumerate(gs):
            if c >= 4 and slot == 2:
                continue  # dummy
            sl = slice(slot * G, (slot + 1) * G)
            out_sorted[g * G:(g + 1) * G, 0:3] = r["rgb"][:, sl].T
            out_sorted[g * G:(g + 1) * G, 3] = r["sig"][0, sl]
    out = np.empty((B, 4), np.float32)
    out[perm] = out_sorted
    return out


def get_nc(reps=1):
    key = f"nc{reps}"
    if key not in _STATE:
        _STATE[key] = _build_nc(reps)
    return _STATE[key]


def kernel(**inputs) -> np.ndarray:
    nc = get_nc()
    in_maps, perm = _prep(inputs)
    res = bass_utils.run_bass_kernel_spmd(nc, in_maps, core_ids=list(range(N_CORES)))
    return _assemble(res.results, perm)



# revision 27
# speedup vs baseline: 3.9818x; 1.0874x over previous
"""Trainium2 Bass kernel for nn_NeRFGraph (gnn_message_passing).

Strategy (sharding_hint): nodes are sharded across 8 cores aligned to whole
knn batch groups. 20 groups of 2048 nodes -> cores 0-3 take 3 groups,
cores 4-7 take 2 real groups + 1 dummy (SPMD needs uniform shapes; dummy
output is dropped on the host). MLP weights are replicated (data parallel).

Per-core pipeline, per group g (layouts are [features(partitions), nodes(free)]):
  1. kNN via ONE bf16 matmul per [128,512] score tile:
     score = a_i.a_j + s1_j + s2_j with a = bf16(x), s1+s2 = hi/lo bf16 split
     of -|x|^2/2. The dropped hi/lo cross terms flip ~500/40960 neighbor
     sets (measured end-to-end 2.2e-3 rel; tolerance 2e-2). Self always wins
     top-1, so neighbors = entries 1..3 of the DVE max8/max_index scan.
     Emission is software-pipelined: tile t's scan is emitted before tile
     t+1's matmuls so the PSUM refill sits ahead of dripped filler work in
     the in-order PE queue.
  2. MLP (8 layers + skip at 4) in float32r, node-pair blocks of 1024;
     thunks dripped evenly between kNN row tiles (PE fills DVE-scan gaps).
  3. EdgeConv x2, factorized: msg_ij = relu(A_i + C_j). A tables on-chip
     (f32); C tables computed TRANSPOSED (per 128-node chunk, stationary =
     feat data, moving = C-weights; all-bf16 matmuls) -> bf16 node-major
     [2048,128] tables staged to HBM, then gathered per edge with SWDGE
     dma_gather (transpose=True, elem_size=128, 512-idx chunks on 4
     rotating queues; queue = emission-index %% 4 to match Tile's DMASW sem
     rotation; chunks of 1024+ idxs overflow the 1024-entry descriptor ring
     and hang the device). This replaced gpsimd ap_gather, which at
     ~30us/1024-idx call was 87%% of runtime. W2 matmul + mean over K=3
     folded into next layer's weights (host prescale by 1/3). EdgeConv of
     group g is emitted interleaved into group g+1's kNN/MLP emission.
     NOTE (HW erratum): f32r matmuls with moving size < 256 run in a
     replicated multi-pass mode whose accumulation corrupts across PE
     p-state transitions (CoreSim is clean; HW produced ~3x-accumulated
     garbage in exactly the chunks before the 3us ramp boundary). ALL
     narrow-N table matmuls are bf16 for this reason; keep f32r matmuls at
     moving size >= 512. Also: bf16 DVE msg ops and finer drip measured
     SLOWER on HW than f32 despite sim predictions - keep msg path f32 and
     validate every scheduling change by on-device A/B (T(9) vs T(1)
     differencing), not by the simulator.
  4. rgb = sigmoid(S2 @ w_rgb/3 + b_rgb), sigma from the MLP trunk.
"""

import numpy as np
import ml_dtypes

import concourse.bass as bass
import concourse.tile as tile
from concourse import bacc, mybir, library_config
import concourse.bass_utils as bass_utils

F32 = mybir.dt.float32
F32R = mybir.dt.float32r
BF16 = mybir.dt.bfloat16
U16 = mybir.dt.uint16
I16 = mybir.dt.int16
NPBF = ml_dtypes.bfloat16

# problem constants (hardcoded per contract)
B = 40960
NG = 20
DXYZ = 63
DDIR = 27
W = 256
H = 128  # W // 2
KNN = 3

N_CORES = 8
GPC = 3                      # groups per core (SPMD-uniform)
G = B // NG                  # 2048 nodes per group
NODES = GPC * G              # 6144 nodes per core
MT = G // 128                # row tiles of 128 per group (knn)
NP = G // 1024               # node pairs of 1024 per group (mlp)

_STATE: dict = {}


def _build_nc(reps=1):
    nc = bacc.Bacc(
        "TRN2",
        target_bir_lowering=False,
        debug=False,
        enable_asserts=False,
        num_devices=N_CORES,
        num_swdge_queues=4,
    )
    d = {}

    def inp(name, shape, dt=F32):
        d[name] = nc.dram_tensor(name, list(shape), dt, kind="ExternalInput").ap()

    inp("xt", (91, NODES), F32R)    # rows 0-62 xyz, 63 ones, 64-90 dir (f32 bits)
    inp("ks", (66, NODES), BF16)    # [a; 1; 1; 1] stationary (pad to 66)
    inp("mb", (66, NODES), BF16)    # [a; s1; s2; 0] moving
    inp("xdb", (DDIR, NODES), BF16)  # dir rows (bf16 copy for table matmuls)
    inp("w0", (DXYZ, W)); inp("b0", (W, 1))
    inp("wmid", (6, W, W))          # [layer, in, out]
    inp("bmid", (6, W, 1))
    inp("wskip", (DXYZ + W, W)); inp("bskip", (W, 1))
    inp("wfin", (W, W)); inp("bfin", (W, 1))
    inp("wsig", (W, 1)); inp("bsig", (1, 1))
    inp("naw1", (W + DDIR, H)); inp("cw1", (W + DDIR, H)); inp("nab1", (H, 1))
    inp("e1w2", (H, H)); inp("e1b2", (H, 1))
    inp("na2w", (H, H)); inp("c2w", (H, H)); inp("nab2", (H, 1))
    inp("e2w2", (H, H)); inp("e2b2", (H, 1))
    inp("wrgb", (H, 3)); inp("brgb", (3, 1))

    rgb_d = nc.dram_tensor("rgb", [3, NODES], F32, kind="ExternalOutput").ap()

    sig_d = nc.dram_tensor("sig", [1, NODES], F32, kind="ExternalOutput").ap()

    with tile.TileContext(nc) as tc:
        _body(tc, d, rgb_d, sig_d, reps=reps)
    nc.compile()
    return nc


def _body(tc, d, rgb_d, sig_d, reps=1):
    nc = tc.nc
    ctxs = []

    def pool(name, bufs, space="SBUF"):
        p = tc.tile_pool(name=name, bufs=bufs, space=space)
        ctxs.append(p)
        return p.__enter__()

    wstage = pool("wstage", bufs=1)       # f32 staging for weight rounding
    wp = pool("wp", bufs=1)               # persistent rounded weights / biases
    xp = pool("xp", bufs=2)               # per-group xt (f32r)
    xk = pool("xk", bufs=1)               # knn bf16 inputs
    ap2 = pool("ap2", bufs=2)             # per-group na1/c1 gather tables
    ap1 = pool("ap1", bufs=1)             # per-group s1/na2/c2 tables
    ec = pool("ec", bufs=2)               # edge-conv small chunk tiles
    ecb = pool("ecb", bufs=2)             # edge-conv big msg tiles (Pool-only)
    ecg = pool("ecg", bufs=2)             # full-group gather outputs [128,3,G]
    hp = pool("hp", bufs=2)               # MLP hidden tiles [128,1024]
    fp = pool("fp", bufs=1)               # feat tiles [128,1024]
    sp = pool("sp", bufs=6)               # small tiles (vmax/imax)
    op = pool("op", bufs=1)               # output staging
    idxp = pool("idxp", bufs=2)
    psp = pool("psp", bufs=1, space="PSUM")    # knn scores [128,2048]
    psm = pool("psm", bufs=2, space="PSUM")    # everything else [128,1024]
    drp = pool("drp", bufs=2, space="DRAM")

    nc.gpsimd.load_library(library_config.mlp)
    # SWDGE queue assignment must match Tile's DMASW sem rotation (8 sems,
    # round-robin): queue = (emission index of Pool-engine DMA insts) % 4.
    swq = [0]

    def nextq():
        q = swq[0] % 4
        swq[0] += 1
        return q

    # ---- per-group input loads (group 0 emitted BEFORE the weight loads so
    # its DMAs are first in the HWDGE queues and kNN can start immediately) ----
    def load_group(g):
        g0 = g * G
        t = {}
        t["xt"] = xp.tile([91, G], F32R, tag="xt", name="xt")
        nc.sync.dma_start(t["xt"][:], d["xt"][:, g0:g0 + G])
        for nm, rows in (("ks", 66), ("mb", 66), ("xdb", DDIR)):
            t[nm] = xk.tile([rows, G], BF16, tag=nm, name=nm)
            nc.sync.dma_start(t[nm][:], d[nm][:, g0:g0 + G])
        return t

    _pre0 = load_group(0)

    # ---- load + round weights to f32r (one-time) ----
    def load_chunks(src_ap, rows, cols, tag, part_off=0, dt=F32R):
        """src_ap: DRAM AP [R, cols]; returns list of rounded chunk tile APs.
        part_off: place the LAST chunk at this base partition (32-aligned)."""
        out = []
        r0 = 0
        for i, r in enumerate(rows):
            last = i == len(rows) - 1
            if last and part_off:
                st = wstage.tile([part_off + r, cols], F32, tag="wstage_p", name="stp")
                nc.sync.dma_start(st[part_off:part_off + r, :], src_ap[r0:r0 + r, :])
                wt = wp.tile([part_off + r, cols], dt, tag=f"{tag}_{i}", name="wtp")
                nc.scalar.activation(wt[part_off:part_off + r, :],
                                     st[part_off:part_off + r, :],
                                     mybir.ActivationFunctionType.Identity)
                out.append(wt[part_off:part_off + r, :])
            else:
                st = wstage.tile([r, cols], F32, tag="wstage", name="st")
                nc.sync.dma_start(st[:], src_ap[r0:r0 + r, :])
                wt = wp.tile([r, cols], dt, tag=f"{tag}_{i}", name="wt")
                nc.scalar.activation(wt[:], st[:], mybir.ActivationFunctionType.Identity)
                out.append(wt[:])
            r0 += r
        return out

    def load_b(name, src_ap, rows):
        out = []
        r0 = 0
        for i, r in enumerate(rows):
            bt = wp.tile([r, 1], F32, tag=f"{name}_{i}", name="bt")
            nc.sync.dma_start(bt[:], src_ap[r0:r0 + r, :])
            out.append(bt)
            r0 += r
        return out

    w0 = load_chunks(d["w0"][:], [DXYZ], W, "w0")[0]
    wmid = [load_chunks(d["wmid"][l], [128, 128], W, f"wmid{l}") for l in range(6)]
    wskip = load_chunks(d["wskip"][:], [DXYZ, 128, 128], W, "wskip")
    wfin = load_chunks(d["wfin"][:], [128, 128], W, "wfin")
    wsig = load_chunks(d["wsig"][:], [128, 128], 1, "wsig")
    naw1 = load_chunks(d["naw1"][:], [128, 128, DDIR], H, "naw1", dt=BF16)
    cw1 = load_chunks(d["cw1"][:], [128, 128, DDIR], H, "cw1", dt=BF16)
    e1w2 = load_chunks(d["e1w2"][:], [H], H, "e1w2")[0]
    na2w = load_chunks(d["na2w"][:], [H], H, "na2w", dt=BF16)[0]
    c2w = load_chunks(d["c2w"][:], [H], H, "c2w", dt=BF16)[0]
    e2w2 = load_chunks(d["e2w2"][:], [H], H, "e2w2")[0]
    wrgb = load_chunks(d["wrgb"][:], [H], 3, "wrgb")[0]

    b0 = load_b("b0", d["b0"][:], [128, 128])
    bmid = [load_b(f"bmid{l}", d["bmid"][l], [128, 128]) for l in range(6)]
    bskip = load_b("bskip", d["bskip"][:], [128, 128])
    bfin = load_b("bfin", d["bfin"][:], [128, 128])
    bsig = load_b("bsig", d["bsig"][:], [1])[0]
    nab1 = load_b("nab1", d["nab1"][:], [H])[0]
    e1b2 = load_b("e1b2", d["e1b2"][:], [H])[0]
    nab2 = load_b("nab2", d["nab2"][:], [H])[0]
    e2b2 = load_b("e2b2", d["e2b2"][:], [H])[0]
    brgb = load_b("brgb", d["brgb"][:], [3])[0]

    ACT = mybir.ActivationFunctionType

    pending = []
    for gi in range(GPC * reps):
        g = gi % GPC
        g0 = g * G
        t_in = _pre0 if gi == 0 else load_group(g)
        xt = t_in["xt"]
        ks = t_in["ks"]; mb = t_in["mb"]; xdb = t_in["xdb"]

        nbr = drp.tile([KNN * G], U16, tag="nbr")   # wrapped k-major image
        # addr = k*2048 + r*128 + q  <->  element m of gather list k at [r=m%16, f=m//16]
        nbr3 = nbr[:].rearrange("(k r q) -> q r k", k=KNN, r=16, q=128)
        na1 = ap2.tile([H, G], F32, tag="na1")
        ct1 = drp.tile([G, H], BF16, tag="ct1")

        def knn_mm(mt):
            ps = psp.tile([128, 2048], F32, tag="ps", name="ps")
            msl = slice(mt * 128, (mt + 1) * 128)
            for nt in range(4):
                osl = slice(nt * 512, (nt + 1) * 512)
                nc.tensor.matmul(ps[:, osl], ks[:, msl], mb[:, osl],
                                 start=True, stop=True)
            return ps

        def knn_scan(mt, ps):
            vmax = sp.tile([128, 8], F32, tag="vmax")
            nc.vector.max(vmax[:], ps[:])
            imax = sp.tile([128, 8], U16, tag="imax")
            nc.vector.max_index(imax[:], vmax[:], ps[:])
            for k in range(KNN):
                nc.sync.dma_start(nbr3[mt * 8:(mt + 1) * 8, :, k], imax[:, 1 + k])

        def mlp_np_thunks(p):
            """MLP trunk for node-pair block p, as a list of per-layer thunks
            so PE work can be dripped between kNN row-tiles."""
            n0 = p * 1024
            sl = slice(n0, n0 + 1024)
            hsl = [slice(n0, n0 + 512), slice(n0 + 512, n0 + 1024)]
            lsl = [slice(0, 512), slice(512, 1024)]
            st = {}

            def layer_mms(ps_list, wchunks, movers):
                # ps_list: [tile for ch0, tile for ch1]; movers: list of
                # (moving AP for ns0, moving AP for ns1) per weight chunk
                for ch in range(2):
                    csl = slice(ch * 128, (ch + 1) * 128)
                    for ns in range(2):
                        for i, wk in enumerate(wchunks):
                            nc.tensor.matmul(
                                ps_list[ch][:, lsl[ns]], wk[:, csl], movers[i][ns],
                                start=(i == 0), stop=(i == len(wchunks) - 1))

            def hmov(h):
                return [(h[0][:, lsl[0]], h[0][:, lsl[1]]),
                        (h[1][:, lsl[0]], h[1][:, lsl[1]])]

            def t0():
                ps = [psm.tile([128, 1024], F32, tag="pm", name=f"ps0{ch}") for ch in range(2)]
                layer_mms(ps, [w0], [(xt[0:DXYZ, hsl[0]], xt[0:DXYZ, hsl[1]])])
                h = [hp.tile([128, 1024], F32R, tag=f"h{ch}", name=f"h{ch}") for ch in range(2)]
                for ch in range(2):
                    nc.scalar.activation(h[ch][:], ps[ch][:], ACT.Relu, bias=b0[ch][:])
                st["h"] = h

            def tl(layer, m):
                h = st["h"]
                ps = [psm.tile([128, 1024], F32, tag="pm", name=f"psl{ch}") for ch in range(2)]
                if layer == 4:
                    bk = bskip
                    layer_mms(ps, wskip,
                              [(xt[0:DXYZ, hsl[0]], xt[0:DXYZ, hsl[1]])] + hmov(h))
                else:
                    bk = bmid[m]
                    layer_mms(ps, wmid[m], hmov(h))
                hn = [hp.tile([128, 1024], F32R, tag=f"h{ch}", name=f"hn{ch}") for ch in range(2)]
                for ch in range(2):
                    nc.scalar.activation(hn[ch][:], ps[ch][:], ACT.Relu, bias=bk[ch][:])
                st["h"] = hn

            def tfin():
                movers = hmov(st["h"])
                ps = [psm.tile([128, 1024], F32, tag="pm", name=f"psf{ch}") for ch in range(2)]
                layer_mms(ps, wfin, movers)
                pss = psm.tile([1, 1024], F32, tag="pm", name="pss")
                for ns in range(2):
                    nc.tensor.matmul(pss[0:1, lsl[ns]], wsig[0][:], movers[0][ns],
                                     start=True, stop=False)
                    nc.tensor.matmul(pss[0:1, lsl[ns]], wsig[1][:], movers[1][ns],
                                     start=False, stop=True)
                feat = [fp.tile([128, 1024], BF16, tag=f"feat{ch}", name=f"feat{ch}") for ch in range(2)]
                for ch in range(2):
                    nc.scalar.activation(feat[ch][:], ps[ch][:], ACT.Identity, bias=bfin[ch][:])
                sgt = op.tile([1, 1024], F32, tag="sgt")
                nc.scalar.activation(sgt[:], pss[0:1, :], ACT.Identity, bias=bsig[:])
                nc.sync.dma_start(sig_d[:, g0 + n0:g0 + n0 + 1024], sgt[:])
                st["feat"] = feat

            def ta1c1():
                feat = st["feat"]
                fmov = hmov(feat) + [(xdb[:, hsl[0]], xdb[:, hsl[1]])]
                psA = psm.tile([128, 1024], F32, tag="pm", name="psA")
                for ns in range(2):
                    for i in range(3):
                        nc.tensor.matmul(psA[:, lsl[ns]], naw1[i], fmov[i][ns],
                                         start=(i == 0), stop=(i == 2))
                nc.scalar.activation(na1[:, sl], psA[:], ACT.Identity, bias=nab1[:])
                # C1 transposed: per 128-node chunk, [nodes, feats] = sum_i
                # stationary(feat-data chunk) x moving(cw1 chunk); bf16 out to
                # HBM node-major for the SWDGE gather.
                psT = psm.tile([128, 1024], F32, tag="pm", name="psT")
                for c in range(8):
                    lc = slice(c * 128, (c + 1) * 128)
                    gsl = slice(n0 + c * 128, n0 + (c + 1) * 128)
                    nc.tensor.matmul(psT[:, lc], feat[0][:, lc], cw1[0],
                                     start=True, stop=False)
                    nc.tensor.matmul(psT[:, lc], feat[1][:, lc], cw1[1],
                                     start=False, stop=False)
                    nc.tensor.matmul(psT[:, lc], xdb[:, gsl], cw1[2],
                                     start=False, stop=True)
                ctsb = ec.tile([128, 1024], BF16, tag="ctsb")
                nc.scalar.activation(ctsb[:], psT[:], ACT.Copy)
                nc.sync.dma_start(
                    ct1[n0:n0 + 1024, :].rearrange("(c p) f -> p c f", p=128),
                    ctsb[:].rearrange("p (c f) -> p c f", c=8))

            thunks = [t0]
            m = 0
            for layer in range(1, 8):
                mm = m
                thunks.append(lambda l=layer, mi_=mm: tl(l, mi_))
                if layer != 4:
                    m += 1
            thunks += [tfin, ta1c1]
            return thunks

        # interleave knn row-tiles + MLP layer thunks + the PREVIOUS group's
        # EdgeConv thunks, so every engine's in-order queue alternates
        # between the two groups (software pipeline) and PE work is smooth.
        mlp_thunks = mlp_np_thunks(0) + mlp_np_thunks(1)
        ti = 0
        mi = 0
        nmt = len(mlp_thunks)
        ps_cur = knn_mm(0)
        for mt in range(MT):
            knn_scan(mt, ps_cur)
            if mt + 1 < MT:
                ps_cur = knn_mm(mt + 1)
            if ti < len(pending):
                pending[ti](); ti += 1
            while mi < (mt + 1) * nmt // MT:
                mlp_thunks[mi](); mi += 1
            if mt % 8 == 7 and ti < len(pending):
                pending[ti](); ti += 1
        while mi < nmt:
            mlp_thunks[mi](); mi += 1
        while ti < len(pending):
            pending[ti](); ti += 1

        # wrapped gather indices, k-major: idxw[:, k*128+f] block for neighbor k.
        # One strided DRAM read into partitions 0:16, then replicate to all
        # 16-partition blocks (one per Q7 core) with SBUF->SBUF copies.
        idxw = idxp.tile([128, G * KNN // 16], I16, tag="idxw")
        nbr_r = nbr[:].rearrange("(k r f) -> r k f", k=KNN, r=16, f=128)
        nc.sync.dma_start(
            idxw[0:16, :].rearrange("r (k f) -> r k f", k=KNN),
            nbr_r.bitcast(I16))
        for r in range(1, 8):
            nc.sync.dma_start(idxw[16 * r:16 * r + 16, :], idxw[0:16, :])

        def make_ec_thunks(g0, idxw, na1, ct1):
            """Build the EdgeConv thunk list for this group; emitted later,
            interleaved into the NEXT group's knn/mlp emission."""
            s1 = ap1.tile([H, G], BF16, tag="s1")
            na2 = ap1.tile([H, G], F32, tag="na2")
            ct2 = drp.tile([G, H], BF16, tag="ct2")
            thunks = []

            def conv_thunks(src, nA, w2, b2, dst_of, out_cb):
                gts = {}

                def gather_half(half):
                    gt = ecg.tile([128, KNN, 1024], BF16, tag="g1")
                    gts[half] = gt
                    for k in range(KNN):
                        for h2 in range(2):
                            nc.gpsimd.dma_gather(
                                gt[:, k:k + 1, h2 * 512:(h2 + 1) * 512],
                                src[:], idxw[:, k * 128 + half * 64 + h2 * 32:
                                             k * 128 + half * 64 + h2 * 32 + 32],
                                512, 512, H, transpose=True, queue_num=nextq())

                def do_chunk(c):
                    gt, cc = gts[c // 2], c % 2
                    nsl = slice(c * 512, (c + 1) * 512)
                    nab = nA[:, nsl].unsqueeze(1).to_broadcast([H, KNN, 512])
                    msgp = ecb.tile([128, KNN, 512], F32, tag="tmx")
                    nc.vector.tensor_sub(msgp[:], gt[:, :, cc * 512:(cc + 1) * 512], nab)
                    msgr = ecb.tile([128, KNN, 512], F32R, tag="msgr")
                    nc.scalar.activation(msgr[:], msgp[:], ACT.Relu)
                    mr = msgr[:]
                    psE = psm.tile([128, 1024], F32, tag="pm", name="psE")
                    psE2 = psm.tile([128, 1024], F32, tag="pm", name="psE2")
                    nc.tensor.matmul(psE[:, 0:512], w2[:], mr[:, 0, :], start=True, stop=True)
                    nc.tensor.matmul(psE[:, 512:1024], w2[:], mr[:, 1, :], start=True, stop=True)
                    nc.tensor.matmul(psE2[:, 0:512], w2[:], mr[:, 2, :], start=True, stop=True)
                    h2 = ec.tile([128, 1024], F32, tag="h2")
                    h22 = ec.tile([128, 512], F32, tag="h22")
                    nc.scalar.activation(h2[:], psE[:], ACT.Relu, bias=b2[:])
                    nc.scalar.activation(h22[:], psE2[:, 0:512], ACT.Relu, bias=b2[:])
                    tmp = ec.tile([128, 512], F32, tag="trio")
                    nc.vector.tensor_add(tmp[:], h2[:, 0:512], h2[:, 512:1024])
                    dst = dst_of(c)
                    nc.vector.tensor_add(dst, tmp[:], h22[:])
                    out_cb(c, dst)

                for half in range(2):
                    thunks.append(lambda h=half: gather_half(h))
                    thunks.append(lambda c=half * 2: do_chunk(c))
                    thunks.append(lambda c=half * 2 + 1: do_chunk(c))

            # ---- EdgeConv 1 ----
            conv_thunks(ct1, na1, e1w2, e1b2,
                        lambda c: s1[:, c * 512:(c + 1) * 512], lambda c, dstap: None)

            # ---- A2 / C2 ----
            def a2c2(p):
                lsl = [slice(p * 1024, p * 1024 + 512), slice(p * 1024 + 512, p * 1024 + 1024)]
                s1r = s1[:]
                psA = psm.tile([128, 1024], F32, tag="pm", name="psA2")
                for ns in range(2):
                    nc.tensor.matmul(psA[:, ns * 512:(ns + 1) * 512], na2w[:], s1r[:, lsl[ns]],
                                     start=True, stop=True)
                nc.scalar.activation(na2[:, p * 1024:(p + 1) * 1024], psA[:], ACT.Identity, bias=nab2[:])
                psT = psm.tile([128, 1024], F32, tag="pm", name="psT2")
                for c in range(8):
                    lc = slice(c * 128, (c + 1) * 128)
                    gsl = slice(p * 1024 + c * 128, p * 1024 + (c + 1) * 128)
                    nc.tensor.matmul(psT[:, lc], s1r[:, gsl], c2w[:],
                                     start=True, stop=True)
                ctsb = ec.tile([128, 1024], BF16, tag="ctsb")
                nc.scalar.activation(ctsb[:], psT[:], ACT.Copy)
                nc.sync.dma_start(
                    ct2[p * 1024:(p + 1) * 1024, :].rearrange("(c p) f -> p c f", p=128),
                    ctsb[:].rearrange("p (c f) -> p c f", c=8))

            thunks.append(lambda: a2c2(0))
            thunks.append(lambda: a2c2(1))

            # ---- EdgeConv 2 ----
            s2t = ec.tile([128, 512], F32R, tag="s2final")

            def ec2_out(c, dstap):
                psR = psm.tile([3, 512], F32, tag="pm", name="psR")
                nc.tensor.matmul(psR[0:3, 0:512], wrgb[:], dstap,
                                 start=True, stop=True)
                rgt = op.tile([3, 512], F32, tag="rgt")
                nc.scalar.activation(rgt[:], psR[0:3, 0:512], ACT.Sigmoid, bias=brgb[:])
                nc.sync.dma_start(rgb_d[:, g0 + c * 512:g0 + (c + 1) * 512], rgt[:])

            conv_thunks(ct2, na2, e2w2, e2b2, lambda c: s2t[:], ec2_out)
            return thunks

        pending = make_ec_thunks(g0, idxw, na1, ct1)

    for t in pending:
        t()

    for p in reversed(ctxs):
        p.__exit__(None, None, None)


def _core_groups():
    cg = []
    for c in range(N_CORES):
        if c < 4:
            gs = [3 * c, 3 * c + 1, 3 * c + 2]
        else:
            g0 = 12 + 2 * (c - 4)
            gs = [g0, g0 + 1, g0]  # 3rd slot = dummy repeat
        cg.append(gs)
    return cg


def _prep(inputs):
    x = np.asarray(inputs["x"], dtype=np.float32)
    batch_ids = np.asarray(inputs["batch_ids"])
    perm = np.argsort(batch_ids, kind="stable")
    xs = np.ascontiguousarray(x[perm])

    xyz = xs[:, :DXYZ]
    sq = (xyz * xyz).sum(1, dtype=np.float32)

    w = {k: np.asarray(inputs[k], dtype=np.float32) for k in inputs if k not in ("x", "batch_ids")}
    e1 = w["e1_w1"]
    naw1 = np.ascontiguousarray(-(e1[:W + DDIR] - e1[W + DDIR:]))
    cw1 = np.ascontiguousarray(e1[W + DDIR:])
    e2 = w["e2_w1"]
    na2w = np.ascontiguousarray(-(e2[:H] - e2[H:]) / 3.0)
    c2w = np.ascontiguousarray(e2[H:] / 3.0)

    shared = {
        "w0": w["w0"], "b0": w["b0"].reshape(W, 1),
        "wmid": w["w_mid"], "bmid": w["b_mid"].reshape(6, W, 1),
        "wskip": w["w_skip"], "bskip": w["b_skip"].reshape(W, 1),
        "wfin": w["w_final"], "bfin": w["b_final"].reshape(W, 1),
        "wsig": w["w_sigma"], "bsig": w["b_sigma"].reshape(1, 1),
        "naw1": naw1, "cw1": cw1, "nab1": -w["e1_b1"].reshape(H, 1),
        "e1w2": w["e1_w2"], "e1b2": w["e1_b2"].reshape(H, 1),
        "na2w": na2w, "c2w": c2w, "nab2": -w["e2_b1"].reshape(H, 1),
        "e2w2": w["e2_w2"], "e2b2": w["e2_b2"].reshape(H, 1),
        "wrgb": np.ascontiguousarray(w["w_rgb"] / 3.0), "brgb": w["b_rgb"].reshape(3, 1),
    }
    shared = {k: np.ascontiguousarray(v, dtype=np.float32) for k, v in shared.items()}

    in_maps = []
    for gs in _core_groups():
        rows = np.concatenate([np.arange(g * G, (g + 1) * G) for g in gs])
        xc = xs[rows]
        xyzT = np.ascontiguousarray(xc[:, :DXYZ].T)   # [63, NODES] f32
        xt = np.empty((91, NODES), np.float32)
        xt[0:DXYZ] = xyzT
        xt[DXYZ] = 1.0
        xt[DXYZ + 1:] = xc[:, DXYZ:].T

        a = xyzT.astype(NPBF)
        nh = -0.5 * sq[rows]
        s1 = nh.astype(NPBF)
        s2 = (nh - s1.astype(np.float32)).astype(NPBF)
        ones2 = np.ones((3, NODES), NPBF)
        zeros1 = np.zeros((1, NODES), NPBF)

        m = dict(shared)
        m["xt"] = np.ascontiguousarray(xt)
        m["xdb"] = np.ascontiguousarray(xc[:, DXYZ:].T.astype(NPBF))
        m["ks"] = np.ascontiguousarray(np.concatenate([a, ones2], 0))
        m["mb"] = np.ascontiguousarray(np.concatenate([a, s1[None], s2[None], zeros1], 0))
        in_maps.append(m)
    return in_maps, perm


def _assemble(results, perm):
    out_sorted = np.empty((B, 4), np.float32)
    for c, gs in enumerate(_core_groups()):
        r = results[c]
        for slot, g in enumerate(gs):
            if c >= 4 and slot == 2:
                continue  # dummy
            sl = slice(slot * G, (slot + 1) * G)
            out_sorted[g * G:(g + 1) * G, 0:3] = r["rgb"][:, sl].T
            out_sorted[g * G:(g + 1) * G, 3] = r["sig"][0, sl]
    out = np.empty((B, 4), np.float32)
    out[perm] = out_sorted
    return out


def get_nc(reps=1):
    key = f"nc{reps}"
    if key not in _STATE:
        _STATE[key] = _build_nc(reps)
    return _STATE[key]


def kernel(**inputs) -> np.ndarray:
    nc = get_nc()
    in_maps, perm = _prep(inputs)
    res = bass_utils.run_bass_kernel_spmd(nc, in_maps, core_ids=list(range(N_CORES)))
    return _assemble(res.results, perm)

